# revision 15
# baseline (speedup 1.0000x reference)
"""Trainium2 Bass kernel for nn_Actor (scatter_memory).

Strategy: pure data parallel across 8 NeuronCores (128 samples each, no
collectives). All activations are kept feature-major ([feat_part, cols])
so every linear is lhsT=W.T chunks / rhs=activation chunks. The LSTM uses
batch-major gates (lhsT = x_t/h tiles, rhs = weight rows) so the gate
nonlinearities run as a few wide ACT ops; h is transposed back to
feature-major each step with a PE transpose. The memory module uses
b-major columns (col = b*64 + m) so the softmax over M is local to each
512-column tile (each tile holds 8 complete samples).

Host-side (free) work: input transposes, weight packing, output
re-assembly. Device matmuls run in float32r (full-rate fp32).
"""

import os
import sys

import numpy as np
import ml_dtypes

for _p in ("/opt/trn_rl_repo",):
    if _p not in sys.path:
        sys.path.insert(0, _p)

os.environ.setdefault("JAX_PLATFORMS", "")

import concourse.bass as bass
import concourse.bacc as bacc
import concourse.mybir as mybir
import concourse.tile as tile
from concourse.bass_utils import run_bass_kernel_spmd
from concourse.masks import make_identity

FP32 = mybir.dt.float32
BF16 = mybir.dt.bfloat16
AF = mybir.ActivationFunctionType
ALU = mybir.AluOpType

B, T, M, F = 1024, 64, 64, 256
NCORES = 8
BC = B // NCORES          # 128 samples per core
COLS = T * BC             # 8192 phase-A columns (t-major: col = t*128 + b)
MCOLS = BC * M            # 8192 memory columns (b-major: col = b*64 + m)
NTILE = 512               # free-dim tile
NT_A = COLS // NTILE      # 16
NT_C = MCOLS // NTILE     # 16
SPT = 8                   # samples per 512-col memory tile

# early-fusion input row blocks in the packed xin tensor [1536, 8192]
EF_BLOCKS = [
    ("v", 0, 1),    # vggish   rows 0:128    (1 k-chunk)
    ("e", 1, 4),    # emotion  rows 128:640  (4 k-chunks)
    ("s", 5, 3),    # scene    rows 640:1024 (3 k-chunks, padded 365->384)
    ("p", 8, 4),    # pose     rows 1024:1536
]

# bias_pack column layout (each column is one 128-row chunk, f-major)
BCOL = {}
_c = 0
for _n in ("vgg", "emo", "scn", "pos", "fus", "tmp", "wr1", "f", "i", "c",
           "rd1", "sd", "fin"):
    BCOL[_n] = _c
    _c += 2
BCOL["pre"] = _c  # scalar at partition 0
NBCOL = _c + 1


def _pack(wt):
    """[kin, out] (kin % 128 == 0) -> [128, (kin/128)*out], block j = rows
    128j:128(j+1)."""
    kin, out = wt.shape
    assert kin % 128 == 0
    return np.ascontiguousarray(
        wt.reshape(kin // 128, 128, out).transpose(1, 0, 2).reshape(128, -1)
    ).astype(np.float32)


def _gate_reorder(w_cols, b=None):
    """Reorder gate columns from [i f g o] (torch) to [i f o g]."""
    i, f, g, o = (w_cols[:, k * F:(k + 1) * F] for k in range(4))
    w = np.concatenate([i, f, o, g], axis=1)
    if b is None:
        return w
    bi, bf, bg, bo = (b[k * F:(k + 1) * F] for k in range(4))
    return w, np.concatenate([bi, bf, bo, bg])


def build_nc():
    nc = bacc.Bacc(None)

    d = {}

    def param(name, shape, out=False, dt=FP32):
        d[name] = nc.declare_dram_parameter(name, list(shape), dt, isOutput=out)
        return d[name]

    param("xin", (1536, COLS), dt=BF16)
    param("sub", (F, MCOLS), dt=BF16)
    for n, c in [("vggW", 256), ("emoW", 1024), ("scnW", 768), ("posW", 1024),
                 ("fusW", 2048), ("lfW", 4096), ("lbW", 2048), ("tmpW", 1024),
                 ("wr1s", 512), ("wr1x", 512), ("fWs", 512), ("fWx", 512),
                 ("iWs", 512), ("iWx", 512), ("cWs", 512), ("cWx", 512),
                 ("rd1s", 512), ("rd1x", 512), ("sdW", 512), ("finW", 512),
                 ("w2pack", 6)]:
        param(n, (128, c), dt=BF16)
    param("bpack", (128, NBCOL))
    param("brow", (1, 2048), dt=BF16)
    param("ones", (1, 128), dt=BF16)
    param("mout", (F, MCOLS), out=True, dt=BF16)
    param("yout", (1, BC), out=True)

    def r(ap):
        return ap

    with tile.TileContext(nc) as tc:
        # ---- persistent pools -------------------------------------------
        with (
            tc.tile_pool(name="wp", bufs=1) as wp,
            tc.tile_pool(name="state", bufs=1) as st,
            tc.tile_pool(name="efin", bufs=2) as efin,
            tc.tile_pool(name="actp", bufs=1) as actp,
            tc.tile_pool(name="fusedp", bufs=2) as fusedp,
            tc.tile_pool(name="lsp", bufs=2) as lsp,
            tc.tile_pool(name="subp", bufs=1) as subp,
            tc.tile_pool(name="cp", bufs=2) as cp,
            tc.tile_pool(name="rowp", bufs=1) as rowp,
            tc.tile_pool(name="efps", bufs=2, space="PSUM") as efps,
            tc.tile_pool(name="gps", bufs=2, space="PSUM") as gps,
            tc.tile_pool(name="tps", bufs=2, space="PSUM") as tps,
        ):
            # ---- load weights ------------------------------------------
            w = {}
            for n, c in [("vggW", 256), ("emoW", 1024), ("scnW", 768),
                         ("posW", 1024), ("fusW", 2048), ("lfW", 4096),
                         ("lbW", 2048), ("tmpW", 1024), ("wr1s", 512),
                         ("wr1x", 512), ("fWs", 512), ("fWx", 512),
                         ("iWs", 512), ("iWx", 512), ("cWs", 512),
                         ("cWx", 512), ("rd1s", 512), ("rd1x", 512),
                         ("sdW", 512), ("finW", 512), ("w2pack", 6)]:
                w[n] = wp.tile([128, c], BF16, name=f"w_{n}")
                nc.sync.dma_start(out=w[n][:, :], in_=d[n][:, :])
            w["bpack"] = wp.tile([128, NBCOL], FP32, name="w_bpack")
            nc.sync.dma_start(out=w["bpack"][:, :], in_=d["bpack"][:, :])
            brow = wp.tile([1, 2048], BF16, name="w_brow")
            nc.sync.dma_start(out=brow[:, :], in_=d["brow"][:, :])
            ones1 = wp.tile([1, 128], BF16, name="ones1")
            nc.sync.dma_start(out=ones1[:, :], in_=d["ones"][:, :])
            ident = wp.tile([128, 128], FP32, name="ident")
            make_identity(nc, ident[:, :])
            bp = w["bpack"]

            def bcol(name, oc=0):
                c0 = BCOL[name] + oc
                return bp[:, c0:c0 + 1]

            # ---- persistent state --------------------------------------
            hT = st.tile([128, 256], BF16, name="hT")        # h fwd, f-major
            c_bm = st.tile([128, 256], FP32, name="c_bm")    # c, batch-major
            nc.vector.memset(hT[:, :], 0.0)
            nc.vector.memset(c_bm[:, :], 0.0)
            sv_un = st.tile([128, 256], FP32, name="sv_un")  # unnorm sub_vec
            s2row = st.tile([1, BC], FP32, name="s2row")     # sum exp(z2) per b

            # ---- helper: feature-major linear into psum ----------------
            def fm_linear(ps_ap, wtile, kparts, rhs_list, n_free):
                """ps = sum_k W_blk[k].T @ rhs[k]; W packed [128, kparts*osz]."""
                nk = len(rhs_list)
                for kc in range(nk):
                    nc.tensor.matmul(
                        ps_ap,
                        lhsT=r(wtile[:, kc * kparts[0] + kparts[1]:
                                     kc * kparts[0] + kparts[1] + 128]),
                        rhs=r(rhs_list[kc]),
                        start=(kc == 0), stop=(kc == nk - 1),
                    )

            # =============================================================
            # Phase A (early fusion) interleaved with LSTM steps
            # =============================================================
            last_fused = None
            for n in range(NT_A):
                cs = slice(n * NTILE, (n + 1) * NTILE)
                xin_t = efin.tile([128, 12, NTILE], BF16, name="xin_t")
                for j in range(12):
                    nc.sync.dma_start(
                        out=xin_t[:, j], in_=d["xin"][j * 128:(j + 1) * 128, cs])
                # per-input linears + relu
                acts = {}
                ei = 0
                for name, blk0, nk in EF_BLOCKS:
                    for oc in range(2):
                        ps = efps.tile([128, NTILE], FP32, name="ps_ef",
                                       tag="efps")
                        wt = w[{"v": "vggW", "e": "emoW", "s": "scnW",
                                "p": "posW"}[name]]
                        for kc in range(nk):
                            nc.tensor.matmul(
                                ps[:, :],
                                lhsT=r(wt[:, kc * 256 + oc * 128:
                                          kc * 256 + oc * 128 + 128]),
                                rhs=r(xin_t[:, blk0 + kc]),
                                start=(kc == 0), stop=(kc == nk - 1))
                        at = actp.tile([128, NTILE], BF16, name=f"act_{name}{oc}",
                                       tag=f"act_{name}{oc}")
                        bc = bcol({"v": "vgg", "e": "emo", "s": "scn",
                                   "p": "pos"}[name], oc)
                        if ei % 2 == 0:
                            nc.scalar.activation(at[:, :], ps[:, :], AF.Relu,
                                                 bias=bc)
                        else:
                            nc.vector.tensor_scalar(
                                out=at[:, :], in0=ps[:, :], scalar1=bc,
                                scalar2=0.0, op0=ALU.add, op1=ALU.max)
                        acts[(name, oc)] = at
                        ei += 1
                # fusion linear + relu -> fused tile (2 chunks)
                fused = []
                for oc in range(2):
                    ps = efps.tile([128, NTILE], FP32, name="ps_fus", tag="efps")
                    mi = 0
                    for ai, (name, _, _) in enumerate(EF_BLOCKS):
                        for kc in range(2):
                            nc.tensor.matmul(
                                ps[:, :],
                                lhsT=r(w["fusW"][:, (ai * 2 + kc) * 256 + oc * 128:
                                                 (ai * 2 + kc) * 256 + oc * 128 + 128]),
                                rhs=r(acts[(name, kc)][:, :]),
                                start=(mi == 0), stop=(mi == 7))
                            mi += 1
                    ft = fusedp.tile([128, NTILE], BF16, name=f"fused{oc}",
                                     tag=f"fused{oc}")
                    if oc == 0:
                        nc.scalar.activation(ft[:, :], ps[:, :], AF.Relu,
                                             bias=bcol("fus", oc))
                    else:
                        nc.vector.tensor_scalar(
                            out=ft[:, :], in0=ps[:, :], scalar1=bcol("fus", oc),
                            scalar2=0.0, op0=ALU.add, op1=ALU.max)
                    fused.append(ft)
                last_fused = fused

                # ---- LSTM steps 4n .. 4n+3 -----------------------------
                for tt in range(4):
                    t4 = tt * 128
                    gp = gps.tile([128, 1024], FP32, name="gp", tag="g")
                    for nh in range(2):
                        out_ap = gp[:, nh * 512:(nh + 1) * 512]
                        nc.tensor.matmul(out_ap, lhsT=ones1[:, :],
                                         rhs=brow[0:1, nh * 512:(nh + 1) * 512],
                                         start=True, stop=False)
                        for kc in range(4):
                            if kc < 2:
                                lhs = fused[kc][:, t4:t4 + 128]
                            else:
                                lhs = hT[:, (kc - 2) * 128:(kc - 1) * 128]
                            nc.tensor.matmul(
                                out_ap, lhsT=lhs,
                                rhs=w["lfW"][:, kc * 1024 + nh * 512:
                                             kc * 1024 + nh * 512 + 512],
                                start=False, stop=(kc == 3))
                    gifo = lsp.tile([128, 768], FP32, name="gifo", tag="gifo", bufs=1)
                    nc.scalar.activation(gifo[:, :], gp[:, 0:768], AF.Sigmoid)
                    gt = lsp.tile([128, 256], FP32, name="gt", tag="gt", bufs=1)
                    nc.scalar.activation(gt[:, :], gp[:, 768:1024], AF.Tanh)
                    t1 = lsp.tile([128, 256], FP32, name="t1", tag="t1", bufs=1)
                    nc.vector.tensor_mul(t1[:, :], gifo[:, 0:256], gt[:, :])
                    nc.vector.tensor_mul(c_bm[:, :], c_bm[:, :], gifo[:, 256:512])
                    nc.vector.tensor_add(c_bm[:, :], c_bm[:, :], t1[:, :])
                    tct = lsp.tile([128, 256], FP32, name="tct", tag="tct", bufs=1)
                    nc.scalar.activation(tct[:, :], c_bm[:, :], AF.Tanh)
                    h_bm = lsp.tile([128, 256], FP32, name="h_bm", tag="h_bm", bufs=1)
                    nc.vector.tensor_mul(h_bm[:, :], gifo[:, 512:768], tct[:, :])
                    tp = tps.tile([128, 256], FP32, name="tp", tag="tp")
                    nc.tensor.transpose(tp[:, 0:128], h_bm[:, 0:128], ident[:, :])
                    nc.tensor.transpose(tp[:, 128:256], h_bm[:, 128:256],
                                        ident[:, :])
                    nc.scalar.copy(hT[:, 0:128], tp[:, 0:128])
                    nc.vector.tensor_copy(hT[:, 128:256], tp[:, 128:256])

            # =============================================================
            # backward cell (t = T-1) + x = relu(tmp([h_fwd; h_bwd]))
            # =============================================================
            gp = gps.tile([128, 1024], FP32, name="gp_b", tag="g")
            for nh in range(2):
                out_ap = gp[:, nh * 512:(nh + 1) * 512]
                nc.tensor.matmul(out_ap, lhsT=ones1[:, :],
                                 rhs=brow[0:1, 1024 + nh * 512:
                                          1024 + (nh + 1) * 512],
                                 start=True, stop=False)
                for kc in range(2):
                    nc.tensor.matmul(
                        out_ap, lhsT=last_fused[kc][:, 384:512],
                        rhs=w["lbW"][:, kc * 1024 + nh * 512:
                                     kc * 1024 + nh * 512 + 512],
                        start=False, stop=(kc == 1))
            gifo = lsp.tile([128, 768], FP32, name="gifo_b", tag="gifo", bufs=1)
            nc.scalar.activation(gifo[:, :], gp[:, 0:768], AF.Sigmoid)
            gt = lsp.tile([128, 256], FP32, name="gt_b", tag="gt", bufs=1)
            nc.scalar.activation(gt[:, :], gp[:, 768:1024], AF.Tanh)
            cb = lsp.tile([128, 256], FP32, name="cb", tag="t1", bufs=1)
            nc.vector.tensor_mul(cb[:, :], gifo[:, 0:256], gt[:, :])
            tcb = lsp.tile([128, 256], FP32, name="tcb", tag="tct", bufs=1)
            nc.scalar.activation(tcb[:, :], cb[:, :], AF.Tanh)
            hb = lsp.tile([128, 256], FP32, name="hb", tag="h_bm", bufs=1)
            nc.vector.tensor_mul(hb[:, :], gifo[:, 512:768], tcb[:, :])
            tp = tps.tile([128, 256], FP32, name="tp_b", tag="tp")
            nc.tensor.transpose(tp[:, 0:128], hb[:, 0:128], ident[:, :])
            nc.tensor.transpose(tp[:, 128:256], hb[:, 128:256], ident[:, :])
            hbT = st.tile([128, 256], BF16, name="hbT")
            nc.scalar.copy(hbT[:, 0:128], tp[:, 0:128])
            nc.vector.tensor_copy(hbT[:, 128:256], tp[:, 128:256])

            # x (feature-major [256, 128]) = relu(tmpW.T @ [hT; hbT] + b)
            xps = tps.tile([128, 256], FP32, name="xps", tag="tp")
            for oc in range(2):
                for kc in range(4):
                    rhs = hT[:, kc * 128:kc * 128 + 128] if kc < 2 else \
                        hbT[:, (kc - 2) * 128:(kc - 2) * 128 + 128]
                    nc.tensor.matmul(
                        xps[:, oc * 128:(oc + 1) * 128],
                        lhsT=r(w["tmpW"][:, kc * 256 + oc * 128:
                                         kc * 256 + oc * 128 + 128]),
                        rhs=r(rhs), start=(kc == 0), stop=(kc == 3))
            x_fm = st.tile([128, 256], BF16, name="x_fm")
            for oc in range(2):
                nc.scalar.activation(x_fm[:, oc * 128:(oc + 1) * 128],
                                     xps[:, oc * 128:(oc + 1) * 128],
                                     AF.Relu, bias=bcol("tmp", oc))

            # q = Wx.T @ x + b for the 5 cat-linears (feature-major [256,128])
            qt = {}
            for qn, wn, bn in [("qwr", "wr1x", "wr1"), ("qf", "fWx", "f"),
                               ("qi", "iWx", "i"), ("qc", "cWx", "c"),
                               ("qrd", "rd1x", "rd1")]:
                ps = tps.tile([128, 256], FP32, name=f"ps_{qn}", tag="tp")
                for oc in range(2):
                    for kc in range(2):
                        nc.tensor.matmul(
                            ps[:, oc * 128:(oc + 1) * 128],
                            lhsT=r(w[wn][:, kc * 256 + oc * 128:
                                         kc * 256 + oc * 128 + 128]),
                            rhs=r(x_fm[:, kc * 128:kc * 128 + 128]),
                            start=(kc == 0), stop=(kc == 1))
                q = st.tile([128, 256], FP32, name=qn)
                for oc in range(2):
                    nc.scalar.activation(q[:, oc * 128:(oc + 1) * 128],
                                         ps[:, oc * 128:(oc + 1) * 128],
                                         AF.Identity, bias=bcol(bn, oc))
                qt[qn] = q

            # =============================================================
            # Memory module, one pass, 16 tiles of 512 cols (8 samples)
            # =============================================================
            def qbc(q, oc, n):
                """q chunk [128, 8 samples] broadcast over 64 m's."""
                return q[:, oc * 128 + n * SPT: oc * 128 + n * SPT + SPT] \
                    .unsqueeze(2).broadcast_to([128, SPT, M])

            for n in range(NT_C):
                cs = slice(n * NTILE, (n + 1) * NTILE)
                sub_t = subp.tile([128, 2, NTILE], BF16, name="sub_t", tag="sub")
                for kc in range(2):
                    nc.sync.dma_start(out=sub_t[:, kc],
                                      in_=d["sub"][kc * 128:(kc + 1) * 128, cs])

                def cat_linear(ws_name, qname, act, bufname, ei):
                    """relu/sig/tanh(Ws.T @ sub + q_bcast) -> [2][128,512]."""
                    outs = []
                    odt = BF16 if act == "relu" else FP32
                    for oc in range(2):
                        ps = efps.tile([128, NTILE], FP32, name=f"ps_{bufname}",
                                       tag="efps")
                        for kc in range(2):
                            nc.tensor.matmul(
                                ps[:, :],
                                lhsT=r(w[ws_name][:, kc * 256 + oc * 128:
                                                  kc * 256 + oc * 128 + 128]),
                                rhs=r(sub_t[:, kc]),
                                start=(kc == 0), stop=(kc == 1))
                        ot = cp.tile([128, NTILE], odt, name=f"{bufname}{oc}",
                                     tag=f"{bufname}{oc}", bufs=1)
                        nc.vector.tensor_add(
                            ot[:, :].rearrange("p (b m) -> p b m", b=SPT),
                            ps[:, :].rearrange("p (b m) -> p b m", b=SPT),
                            qbc(qt[qname], oc, n))
                        if act == "relu":
                            if ei % 2 == 0:
                                nc.scalar.activation(
                                    ot[:, :], ot[:, :], AF.Relu)
                            else:
                                nc.vector.tensor_scalar(
                                    out=ot[:, :], in0=ot[:, :],
                                    scalar1=0.0, scalar2=None, op0=ALU.max)
                        elif act == "sig":
                            nc.scalar.activation(ot[:, :], ot[:, :], AF.Sigmoid)
                        else:
                            nc.scalar.activation(ot[:, :], ot[:, :], AF.Tanh)
                        outs.append(ot)
                    return outs

                r1 = cat_linear("wr1s", "qwr", "relu", "r1", 0)
                # z1 = wr2 . r1  -> [1, 512]
                zp = gps.tile([1, NTILE], FP32, name="zp", tag="g")
                for kc in range(2):
                    nc.tensor.matmul(zp[:, :], lhsT=r(w["w2pack"][:, kc:kc + 1]),
                                     rhs=r(r1[kc][:, :]),
                                     start=(kc == 0), stop=(kc == 1))
                ex1 = rowp.tile([1, NTILE], FP32, name="ex1", tag="ex1")
                nc.scalar.activation(ex1[:, :], zp[:, :], AF.Exp)
                s1 = rowp.tile([1, SPT], FP32, name="s1", tag="s1")
                nc.vector.tensor_reduce(
                    s1[:, :], ex1[:, :].rearrange("p (b m) -> p b m", b=SPT),
                    axis=mybir.AxisListType.X, op=ALU.add)
                rc1 = rowp.tile([1, SPT], FP32, name="rc1", tag="rc1")
                nc.vector.reciprocal(rc1[:, :], s1[:, :])
                nc.vector.tensor_mul(
                    ex1[:, :].rearrange("p (b m) -> p b m", b=SPT),
                    ex1[:, :].rearrange("p (b m) -> p b m", b=SPT),
                    rc1[:, :].unsqueeze(2).broadcast_to([1, SPT, M]))
                simb_t = cp.tile([128, NTILE], FP32, name="simb_t",
                                  tag="bcast", bufs=1)
                nc.gpsimd.partition_broadcast(simb_t[:, :], ex1[:, :])
                simb = simb_t[:, :]

                f_t = cat_linear("fWs", "qf", "sig", "ft", 0)
                i_t = cat_linear("iWs", "qi", "sig", "it", 0)
                c_t = cat_linear("cWs", "qc", "tanh", "ct", 0)

                # mem_pre = sub - sim*(f*sub - i*c), computed in place:
                # f_t <- f*sub; i_t <- i*c; f_t <- f_t - i_t; f_t <- f_t*sim;
                # sub_t <- sub_t - f_t  (mem_pre lives in sub_t)
                mem_pre = []
                for oc in range(2):
                    sub_f = sub_t[:, oc]
                    nc.vector.tensor_mul(f_t[oc][:, :], f_t[oc][:, :], sub_f)
                    nc.gpsimd.tensor_mul(i_t[oc][:, :], i_t[oc][:, :],
                                         c_t[oc][:, :])
                    nc.vector.tensor_sub(f_t[oc][:, :], f_t[oc][:, :],
                                         i_t[oc][:, :])
                    nc.vector.tensor_mul(f_t[oc][:, :], f_t[oc][:, :], simb)
                    nc.vector.tensor_sub(sub_t[:, oc], sub_f,
                                         f_t[oc][:, :])
                    mem_pre.append(None)

                # mem = tanh(sdW.T @ mem_pre + sd_b);  DMA out
                mem_t = []
                for oc in range(2):
                    ps = efps.tile([128, NTILE], FP32, name="ps_sd", tag="efps")
                    for kc in range(2):
                        nc.tensor.matmul(
                            ps[:, :],
                            lhsT=r(w["sdW"][:, kc * 256 + oc * 128:
                                            kc * 256 + oc * 128 + 128]),
                            rhs=r(sub_t[:, kc]),
                            start=(kc == 0), stop=(kc == 1))
                    mt = cp.tile([128, NTILE], BF16, name=f"memt{oc}",
                                 tag=f"memt{oc}")
                    nc.scalar.activation(mt[:, :], ps[:, :], AF.Tanh,
                                         bias=bcol("sd", oc))
                    nc.sync.dma_start(out=d["mout"][oc * 128:(oc + 1) * 128, cs],
                                      in_=mt[:, :])
                    mem_t.append(mt)

                # read attention: z2 over relu(rd1s.T @ mem + q_rd)
                r2 = []
                for oc in range(2):
                    ps = efps.tile([128, NTILE], FP32, name="ps_r2", tag="efps")
                    for kc in range(2):
                        nc.tensor.matmul(
                            ps[:, :],
                            lhsT=r(w["rd1s"][:, kc * 256 + oc * 128:
                                             kc * 256 + oc * 128 + 128]),
                            rhs=r(mem_t[kc][:, :]),
                            start=(kc == 0), stop=(kc == 1))
                    ot = cp.tile([128, NTILE], BF16, name=f"r2{oc}",
                                 tag=f"r1{oc}", bufs=1)
                    nc.vector.tensor_add(
                        ot.rearrange("p (b m) -> p b m", b=SPT),
                        ps.rearrange("p (b m) -> p b m", b=SPT),
                        qbc(qt["qrd"], oc, n))
                    if oc == 0:
                        nc.scalar.activation(ot[:, :], ot[:, :],
                                             AF.Relu)
                    else:
                        nc.vector.tensor_scalar(
                            out=ot[:, :], in0=ot[:, :],
                            scalar1=0.0, scalar2=None, op0=ALU.max)
                    r2.append(ot)
                zp2 = gps.tile([1, NTILE], FP32, name="zp2", tag="g")
                for kc in range(2):
                    nc.tensor.matmul(zp2[:, :],
                                     lhsT=r(w["w2pack"][:, 2 + kc:3 + kc]),
                                     rhs=r(r2[kc][:, :]),
                                     start=(kc == 0), stop=(kc == 1))
                ex2 = rowp.tile([1, NTILE], FP32, name="ex2", tag="ex1")
                nc.scalar.activation(ex2[:, :], zp2[:, :], AF.Exp)
                nc.vector.tensor_reduce(
                    s2row[:, n * SPT:(n + 1) * SPT],
                    ex2[:, :].rearrange("p (b m) -> p b m", b=SPT),
                    axis=mybir.AxisListType.X, op=ALU.add)
                ex2b_t = cp.tile([128, NTILE], FP32, name="ex2b_t",
                                   tag="bcast", bufs=1)
                nc.gpsimd.partition_broadcast(ex2b_t[:, :], ex2[:, :])
                ex2b = ex2b_t[:, :]
                for oc in range(2):
                    wm = c_t[oc]
                    nc.vector.tensor_mul(wm[:, :], mem_t[oc][:, :], ex2b)
                    nc.vector.tensor_reduce(
                        sv_un[:, oc * 128 + n * SPT: oc * 128 + (n + 1) * SPT],
                        wm[:, :].rearrange("p (b m) -> p b m", b=SPT),
                        axis=mybir.AxisListType.X, op=ALU.add)

            # ---- tail: sub_vec, final, y -------------------------------
            rc2 = rowp.tile([1, BC], FP32, name="rc2", tag="rc2")
            nc.vector.reciprocal(rc2[:, :], s2row[:, :])
            rc2b_t = cp.tile([128, BC], FP32, name="rc2b_t",
                              tag="bcast", bufs=1)
            nc.gpsimd.partition_broadcast(rc2b_t[:, :], rc2[:, :])
            rc2b = rc2b_t[:, :]
            for oc in range(2):
                nc.vector.tensor_mul(sv_un[:, oc * 128:(oc + 1) * 128],
                                     sv_un[:, oc * 128:(oc + 1) * 128], rc2b)
            sv_r = st.tile([128, 256], BF16, name="sv_r")
            nc.scalar.copy(sv_r[:, :], sv_un[:, :])
            fps_ = tps.tile([128, 256], FP32, name="fps", tag="tp")
            for oc in range(2):
                for kc in range(2):
                    nc.tensor.matmul(
                        fps_[:, oc * 128:(oc + 1) * 128],
                        lhsT=r(w["finW"][:, kc * 256 + oc * 128:
                                         kc * 256 + oc * 128 + 128]),
                        rhs=r(sv_r[:, kc * 128:kc * 128 + 128]),
                        start=(kc == 0), stop=(kc == 1))
            fin_t = st.tile([128, 256], BF16, name="fin_t")
            for oc in range(2):
                nc.scalar.activation(fin_t[:, oc * 128:(oc + 1) * 128],
                                     fps_[:, oc * 128:(oc + 1) * 128],
                                     AF.Tanh, bias=bcol("fin", oc))
            yp = gps.tile([1, BC], FP32, name="yp", tag="g")
            for kc in range(2):
                nc.tensor.matmul(yp[:, :], lhsT=r(w["w2pack"][:, 4 + kc:5 + kc]),
                                 rhs=r(fin_t[:, kc * 128:kc * 128 + 128]),
                                 start=(kc == 0), stop=(kc == 1))
            y_t = rowp.tile([1, BC], FP32, name="y_t", tag="y_t")
            nc.scalar.activation(y_t[:, :], yp[:, :], AF.Sigmoid,
                                 bias=bp[0:1, BCOL["pre"]:BCOL["pre"] + 1])
            nc.sync.dma_start(out=d["yout"][:, :], in_=y_t[:, :])

    if not nc.is_finalized():
        nc.finalize()
    return nc


_NC_CACHE = None
LAST_RESULTS = None
LAST_IN_MAPS = None


def _get_nc():
    global _NC_CACHE
    if _NC_CACHE is None:
        _NC_CACHE = build_nc()
    return _NC_CACHE


BFNP = ml_dtypes.bfloat16


def _prep_weights(p):
    """Host-side packing of all parameters -> dict of device arrays."""
    g = {k: np.asarray(v, np.float32) for k, v in p.items()}
    out = {}
    out["vggW"] = _pack(g["vgg_W"].T)
    out["emoW"] = _pack(g["emo_W"].T)
    scn = np.zeros((384, 256), np.float32)
    scn[:365] = g["scn_W"].T
    out["scnW"] = _pack(scn)
    out["posW"] = _pack(g["pos_W"].T)
    out["fusW"] = _pack(g["fus_W"].T)

    lf_cols, lf_b = _gate_reorder(
        np.vstack([g["lf_Wih"].T, g["lf_Whh"].T]), g["lf_b"])
    out["lfW"] = _pack(lf_cols)
    lb_cols, lb_b = _gate_reorder(g["lb_Wih"].T, g["lb_b"])
    out["lbW"] = _pack(lb_cols)
    out["brow"] = np.concatenate([lf_b, lb_b]).reshape(1, 2048).astype(np.float32)
    out["tmpW"] = _pack(g["tmp_W"].T)

    for nm, wn in [("wr1", "wr1_W"), ("f", "f_W"), ("i", "i_W"),
                   ("c", "c_W"), ("rd1", "rd1_W")]:
        key = {"wr1": ("wr1s", "wr1x"), "f": ("fWs", "fWx"),
               "i": ("iWs", "iWx"), "c": ("cWs", "cWx"),
               "rd1": ("rd1s", "rd1x")}[nm]
        out[key[0]] = _pack(g[wn][:, :256].T)
        out[key[1]] = _pack(g[wn][:, 256:].T)
    out["sdW"] = _pack(g["sd_W"].T)
    out["finW"] = _pack(g["fin_W"].T)

    w2 = np.zeros((128, 6), np.float32)
    for j, nm in enumerate(["wr2_W", "rd2_W", "pre_W"]):
        w2[:, 2 * j:2 * j + 2] = g[nm].reshape(256).reshape(2, 128).T
    out["w2pack"] = w2

    bpk = np.zeros((128, NBCOL), np.float32)
    for nm, bn in [("vgg", "vgg_b"), ("emo", "emo_b"), ("scn", "scn_b"),
                   ("pos", "pos_b"), ("fus", "fus_b"), ("tmp", "tmp_b"),
                   ("wr1", "wr1_b"), ("f", "f_b"), ("i", "i_b"),
                   ("c", "c_b"), ("rd1", "rd1_b"), ("sd", "sd_b"),
                   ("fin", "fin_b")]:
        bpk[:, BCOL[nm]:BCOL[nm] + 2] = g[bn].reshape(2, 128).T
    bpk[0, BCOL["pre"]] = float(np.asarray(g["pre_b"]).reshape(-1)[0])
    for k in out:
        out[k] = out[k].astype(BFNP)
    out["bpack"] = bpk
    return out


def kernel(vggish, bgm_emotion, emotion, scene, pose, sub_memory, target,
           params, epochs, training, **_unused):
    vggish = np.asarray(vggish, np.float32)
    emotion = np.asarray(emotion, np.float32)
    scene = np.asarray(scene, np.float32)
    pose = np.asarray(pose, np.float32)
    sub_memory = np.asarray(sub_memory, np.float32)

    wmaps = _prep_weights(params)
    nc = _get_nc()

    in_maps = []
    for ci in range(NCORES):
        bs = slice(ci * BC, (ci + 1) * BC)
        xin = np.empty((1536, COLS), BFNP)
        # [Bc, T, D] -> [D, T, Bc] -> [D, T*Bc]
        xin[0:128] = vggish[bs].transpose(2, 1, 0).reshape(128, COLS)
        xin[128:640] = emotion[bs].transpose(2, 1, 0).reshape(512, COLS)
        xin[640:1005] = scene[bs].transpose(2, 1, 0).reshape(365, COLS)
        xin[1005:1024] = 0.0
        xin[1024:1536] = pose[bs].transpose(2, 1, 0).reshape(512, COLS)
        sub = np.ascontiguousarray(
            sub_memory[bs].transpose(2, 0, 1).reshape(256, MCOLS)).astype(BFNP)
        m = {"xin": xin, "sub": sub,
             "ones": np.ones((1, 128), BFNP)}
        m.update(wmaps)
        in_maps.append(m)

    global LAST_RESULTS, LAST_IN_MAPS
    LAST_IN_MAPS = in_maps
    res = run_bass_kernel_spmd(nc, in_maps, list(range(NCORES)))
    LAST_RESULTS = res

    mem = np.empty((B, M, F), np.float32)
    y = np.empty((B, 1), np.float32)
    for ci in range(NCORES):
        bs = slice(ci * BC, (ci + 1) * BC)
        mo = res.results[ci]["mout"]           # [256, 8192] b-major cols
        mem[bs] = mo.reshape(256, BC, M).transpose(1, 2, 0).astype(np.float32)
        y[bs] = res.results[ci]["yout"].reshape(BC, 1)
    return mem, y


if __name__ == "__main__":
    print("building nc...")
    nc = _get_nc()
    print("ok")


# revision 22
# speedup vs baseline: 1.1919x; 1.1919x over previous
"""Trainium2 Bass kernel for nn_Actor (scatter_memory).

Pure data parallel across 8 NeuronCores (128 samples each, no
collectives). Activations are feature-major ([feat_part, cols]); every
linear is lhsT=W.T chunks / rhs=activation chunks in bf16 (fp32 PSUM
accumulate). The LSTM uses batch-major gates (lhsT = x_t/h tiles,
rhs = weight rows); h returns to feature-major via PE transpose each
step. Memory-module columns are b-major (col = b*64 + m) so softmax
over M is local to each 512-column tile.

All ACT transcendentals use the sigmoid table: tanh(x) = 2*sigmoid(2x)-1
with the 2x folded into host-scaled weights and the affine folded into
neighbouring DVE ops or downstream weights (rd1/pre) or the host-side
output transform (mem = 2*s - 1).
"""

import os
import sys

import numpy as np
import ml_dtypes

for _p in ("/opt/trn_rl_repo",):
    if _p not in sys.path:
        sys.path.insert(0, _p)

os.environ.setdefault("JAX_PLATFORMS", "")

import concourse.bass as bass
import concourse.bacc as bacc
import concourse.mybir as mybir
import concourse.tile as tile
from concourse.bass_utils import run_bass_kernel_spmd
from concourse.masks import make_identity

FP32 = mybir.dt.float32
BF16 = mybir.dt.bfloat16
AF = mybir.ActivationFunctionType
ALU = mybir.AluOpType
BFNP = ml_dtypes.bfloat16

B, T, M, F = 1024, 64, 64, 256
NCORES = 8
BC = B // NCORES          # 128 samples per core
COLS = T * BC             # phase-A columns (t-major: col = t*128 + b)
MCOLS = BC * M            # memory columns (b-major: col = b*64 + m)
NTILE = 512
NT_A = COLS // NTILE      # 16
NT_C = MCOLS // NTILE     # 16
SPT = 8                   # samples per 512-col memory tile

EF_BLOCKS = [
    ("v", 0, 1),    # vggish   rows 0:128
    ("e", 1, 4),    # emotion  rows 128:640
    ("s", 5, 3),    # scene    rows 640:1024 (365 padded to 384)
    ("p", 8, 4),    # pose     rows 1024:1536
]

BCOL = {}
_c = 0
for _n in ("vgg", "emo", "scn", "pos", "fus", "tmp", "wr1", "f", "i", "c",
           "rd1", "sd", "fin"):
    BCOL[_n] = _c
    _c += 2
BCOL["pre"] = _c
NBCOL = _c + 1


def _pack(wt):
    """[kin, out] (kin % 128 == 0) -> [128, (kin/128)*out]."""
    kin, out = wt.shape
    assert kin % 128 == 0
    return np.ascontiguousarray(
        wt.reshape(kin // 128, 128, out).transpose(1, 0, 2).reshape(128, -1))


def _gate_reorder(w_cols, b):
    """[i f g o] -> [i f o g]; double the g block (tanh-as-sigmoid)."""
    i, f, g, o = (w_cols[:, k * F:(k + 1) * F] for k in range(4))
    w = np.concatenate([i, 2.0 * g, f, o], axis=1)
    bi, bf, bg, bo = (b[k * F:(k + 1) * F] for k in range(4))
    return w, np.concatenate([bi, 2.0 * bg, bf, bo])


def build_nc():
    nc = bacc.Bacc(None)
    d = {}

    def param(name, shape, out=False, dt=FP32):
        d[name] = nc.declare_dram_parameter(name, list(shape), dt, isOutput=out)
        return d[name]

    param("xin", (1536, COLS), dt=BF16)
    param("sub", (F, MCOLS), dt=BF16)
    WSPEC = [("vggW", 256), ("emoW", 1024), ("scnW", 768), ("posW", 1024),
             ("fusW", 2048), ("lfW", 4096), ("lbW", 2048), ("tmpW", 1024),
             ("wr1s", 512), ("wr1x", 512), ("fWs", 512), ("fWx", 512),
             ("iWs", 512), ("iWx", 512), ("cWs", 512), ("cWx", 512),
             ("rd1s", 512), ("rd1x", 512), ("sdW", 512), ("finW", 512),
             ("w2pack", 6)]
    for n, c in WSPEC:
        param(n, (128, c), dt=BF16)
    param("bpack", (128, NBCOL))
    param("sel8", (8, NTILE), dt=BF16)
    param("qbrow", (1, 1280), dt=BF16)
    param("brow", (1, 2048), dt=BF16)
    param("ones", (1, 128), dt=BF16)
    param("mout", (F, MCOLS), out=True)
    param("yout", (1, BC), out=True)

    with tile.TileContext(nc) as tc:
        with (
            tc.tile_pool(name="wp", bufs=1) as wp,
            tc.tile_pool(name="state", bufs=1) as st,
            tc.tile_pool(name="efin", bufs=2) as efin,
            tc.tile_pool(name="actp", bufs=2) as actp,
            tc.tile_pool(name="fusedp", bufs=3) as fusedp,
            tc.tile_pool(name="lsp", bufs=2) as lsp,
            tc.tile_pool(name="subp", bufs=3) as subp,
            tc.tile_pool(name="cp", bufs=3) as cp,
            tc.tile_pool(name="rowp", bufs=2) as rowp,
            tc.tile_pool(name="efps", bufs=3, space="PSUM") as efps,
            tc.tile_pool(name="gps", bufs=2, space="PSUM") as gps,
            tc.tile_pool(name="tps", bufs=1, space="PSUM") as tps,
        ):
            # ---- weights ----------------------------------------------
            w = {}
            for n, c in WSPEC:
                w[n] = wp.tile([128, c], BF16, name=f"w_{n}")
                nc.sync.dma_start(out=w[n][:, :], in_=d[n][:, :])
            w["bpack"] = wp.tile([128, NBCOL], FP32, name="w_bpack")
            nc.sync.dma_start(out=w["bpack"][:, :], in_=d["bpack"][:, :])
            brow = wp.tile([1, 2048], BF16, name="w_brow")
            nc.sync.dma_start(out=brow[:, :], in_=d["brow"][:, :])
            sel8 = wp.tile([8, NTILE], BF16, name="w_sel8")
            nc.sync.dma_start(out=sel8[:, :], in_=d["sel8"][:, :])
            qbrow = wp.tile([1, 1280], BF16, name="w_qbrow")
            nc.sync.dma_start(out=qbrow[:, :], in_=d["qbrow"][:, :])
            ones1 = wp.tile([1, 128], BF16, name="ones1")
            nc.sync.dma_start(out=ones1[:, :], in_=d["ones"][:, :])
            ident = wp.tile([128, 128], FP32, name="ident")
            make_identity(nc, ident[:, :])
            ident2 = wp.tile([128, 128], FP32, name="ident2")
            make_identity(nc, ident2[:, :])
            nc.scalar.mul(ident2[:, :], ident2[:, :], 2.0)
            bp = w["bpack"]

            def bcol(name, oc=0):
                c0 = BCOL[name] + oc
                return bp[:, c0:c0 + 1]

            # ---- persistent state -------------------------------------
            hT = st.tile([128, 256], BF16, name="hT")
            c_bm = st.tile([128, 256], FP32, name="c_bm")
            nc.vector.memset(hT[:, :], 0.0)
            nc.vector.memset(c_bm[:, :], 0.0)
            sv_un = st.tile([128, 256], FP32, name="sv_un")
            s2row = st.tile([1, BC], FP32, name="s2row")

            # ============================================================
            # Phase A (early fusion) emitted in quarter-tile slices
            # between LSTM steps (fills PE during the step's serial
            # elementwise chain; the PE queue is in-order, so ready
            # matmuls must be emitted ahead of blocking step matmuls).
            # ============================================================
            WNAME = {"v": "vggW", "e": "emoW", "s": "scnW", "p": "posW"}
            BNAME = {"v": "vgg", "e": "emo", "s": "scn", "p": "pos"}

            def ef_input(n, stt_, name, blk0, nk, ei):
                xin_t = stt_["xin"]
                for oc in range(2):
                    ps = efps.tile([128, NTILE], FP32, name="ps_ef",
                                   tag="efps")
                    wt = w[WNAME[name]]
                    for kc in range(nk):
                        nc.tensor.matmul(
                            ps[:, :],
                            lhsT=wt[:, kc * 256 + oc * 128:
                                    kc * 256 + oc * 128 + 128],
                            rhs=xin_t[:, blk0 + kc],
                            start=(kc == 0), stop=(kc == nk - 1))
                    at = actp.tile([128, NTILE], BF16, name=f"act_{name}{oc}",
                                   tag=f"act_{name}{oc}")
                    bc = bcol(BNAME[name], oc)
                    if (ei + oc) % 2 == 0:
                        nc.scalar.activation(at[:, :], ps[:, :], AF.Relu,
                                             bias=bc)
                    else:
                        nc.vector.tensor_scalar(
                            out=at[:, :], in0=ps[:, :], scalar1=bc,
                            scalar2=0.0, op0=ALU.add, op1=ALU.max)
                    stt_["acts"][(name, oc)] = at

            def ef_quarter(n, q, stt_):
                cs = slice(n * NTILE, (n + 1) * NTILE)
                if q == 0:
                    xin_t = efin.tile([128, 12, NTILE], BF16, name="xin_t")
                    stt_["xin"] = xin_t
                    stt_["acts"] = {}
                    for j in range(12):
                        nc.sync.dma_start(
                            out=xin_t[:, j],
                            in_=d["xin"][j * 128:(j + 1) * 128, cs])
                    ef_input(n, stt_, "v", 0, 1, 0)
                elif q == 1:
                    ef_input(n, stt_, "e", 1, 4, 1)
                elif q == 2:
                    ef_input(n, stt_, "s", 5, 3, 0)
                    ef_input(n, stt_, "p", 8, 4, 1)
                else:
                    acts = stt_["acts"]
                    fused = []
                    for oc in range(2):
                        ps = efps.tile([128, NTILE], FP32, name="ps_fus",
                                       tag="efps")
                        mi = 0
                        for ai, (name, _, _) in enumerate(EF_BLOCKS):
                            for kc in range(2):
                                nc.tensor.matmul(
                                    ps[:, :],
                                    lhsT=w["fusW"][:, (ai * 2 + kc) * 256 + oc * 128:
                                                   (ai * 2 + kc) * 256 + oc * 128 + 128],
                                    rhs=acts[(name, kc)][:, :],
                                    start=(mi == 0), stop=(mi == 7))
                                mi += 1
                        ft = fusedp.tile([128, NTILE], BF16,
                                         name=f"fused{oc}", tag=f"fused{oc}")
                        if oc == 0:
                            nc.scalar.activation(ft[:, :], ps[:, :], AF.Relu,
                                                 bias=bcol("fus", oc))
                        else:
                            nc.vector.tensor_scalar(
                                out=ft[:, :], in0=ps[:, :],
                                scalar1=bcol("fus", oc), scalar2=0.0,
                                op0=ALU.add, op1=ALU.max)
                        fused.append(ft)
                    stt_["fused"] = fused

            def lstm_step(tt, fused):
                # gates (batch-major, cols reordered [i g' f o], g' = 2g):
                # s = sigmoid(gp); g~ = 2*s_g - 1
                # c = f*c + 2*(i*s_g) - i ;  h = 2*(o*sigmoid(2c)) - o
                t4 = tt * 128
                gp = gps.tile([128, 1024], FP32, name="gp", tag="g")
                for nh in range(2):
                    out_ap = gp[:, nh * 512:(nh + 1) * 512]
                    nc.tensor.matmul(out_ap, lhsT=ones1[:, :],
                                     rhs=brow[0:1, nh * 512:(nh + 1) * 512],
                                     start=True, stop=False)
                    for kc in range(4):
                        if kc < 2:
                            lhs = fused[kc][:, t4:t4 + 128]
                        else:
                            lhs = hT[:, (kc - 2) * 128:(kc - 1) * 128]
                        nc.tensor.matmul(
                            out_ap, lhsT=lhs,
                            rhs=w["lfW"][:, kc * 1024 + nh * 512:
                                         kc * 1024 + nh * 512 + 512],
                            start=False, stop=(kc == 3))
                gall = lsp.tile([128, 1024], FP32, name="gall", tag="gall")
                nc.scalar.activation(gall[:, 0:512], gp[:, 0:512], AF.Sigmoid)
                nc.scalar.activation(gall[:, 512:1024], gp[:, 512:1024],
                                     AF.Sigmoid)
                gi, gs = gall[:, 0:256], gall[:, 256:512]
                gf, go = gall[:, 512:768], gall[:, 768:1024]
                # t1 = i*(s_g - 0.5); c = f*c + 2*t1
                t1 = lsp.tile([128, 256], FP32, name="t1", tag="t1")
                nc.vector.scalar_tensor_tensor(
                    out=t1[:, :], in0=gs, scalar=-0.5, in1=gi,
                    op0=ALU.add, op1=ALU.mult)
                nc.vector.tensor_mul(c_bm[:, :], c_bm[:, :], gf)
                nc.vector.scalar_tensor_tensor(
                    out=c_bm[:, :], in0=t1[:, :], scalar=2.0,
                    in1=c_bm[:, :], op0=ALU.mult, op1=ALU.add)
                sc = lsp.tile([128, 256], FP32, name="sc", tag="sc")
                nc.scalar.activation(sc[:, :], c_bm[:, :], AF.Sigmoid,
                                     scale=2.0)
                # h = 2*o*(s_c-0.5); the 2x rides the transpose (ident2)
                h_bm = lsp.tile([128, 256], FP32, name="h_bm", tag="h_bm")
                nc.vector.scalar_tensor_tensor(
                    out=h_bm[:, :], in0=sc[:, :], scalar=-0.5, in1=go,
                    op0=ALU.add, op1=ALU.mult)
                tp = tps.tile([128, 256], FP32, name="tp", tag="tp")
                nc.tensor.transpose(tp[:, 0:128], h_bm[:, 0:128], ident2[:, :])
                nc.tensor.transpose(tp[:, 128:256], h_bm[:, 128:256],
                                    ident2[:, :])
                nc.scalar.copy(hT[:, 0:128], tp[:, 0:128])
                nc.vector.tensor_copy(hT[:, 128:256], tp[:, 128:256])

            st_cur = {}
            for q in range(4):
                ef_quarter(0, q, st_cur)
            fused_cur = st_cur["fused"]
            st_nxt = {}
            for t in range(T):
                lstm_step(t % 4, fused_cur)
                n_next = t // 4 + 1
                if n_next < NT_A:
                    ef_quarter(n_next, t % 4, st_nxt)
                    if t % 4 == 3:
                        fused_cur = st_nxt["fused"]
                        st_nxt = {}
            last_fused = fused_cur

            # ============================================================
            # backward cell (t = T-1), then x = relu(tmp([h_fwd; h_bwd]))
            # ============================================================
            gp = gps.tile([128, 1024], FP32, name="gp_b", tag="g")
            for nh in range(2):
                out_ap = gp[:, nh * 512:(nh + 1) * 512]
                nc.tensor.matmul(out_ap, lhsT=ones1[:, :],
                                 rhs=brow[0:1, 1024 + nh * 512:
                                          1024 + (nh + 1) * 512],
                                 start=True, stop=False)
                for kc in range(2):
                    nc.tensor.matmul(
                        out_ap, lhsT=last_fused[kc][:, 384:512],
                        rhs=w["lbW"][:, kc * 1024 + nh * 512:
                                     kc * 1024 + nh * 512 + 512],
                        start=False, stop=(kc == 1))
            gall = lsp.tile([128, 1024], FP32, name="gall_b", tag="gall")
            nc.scalar.activation(gall[:, :], gp[:, :], AF.Sigmoid)
            gi, gs, go = gall[:, 0:256], gall[:, 256:512], gall[:, 768:1024]
            t1 = lsp.tile([128, 256], FP32, name="t1b", tag="t1")
            nc.vector.scalar_tensor_tensor(
                out=t1[:, :], in0=gs, scalar=-0.5, in1=gi,
                op0=ALU.add, op1=ALU.mult)
            scb = lsp.tile([128, 256], FP32, name="scb", tag="t2")
            nc.scalar.activation(scb[:, :], t1[:, :], AF.Sigmoid, scale=4.0)
            hb = lsp.tile([128, 256], FP32, name="hb", tag="h_bm")
            nc.vector.scalar_tensor_tensor(
                out=hb[:, :], in0=scb[:, :], scalar=-0.5, in1=go,
                op0=ALU.add, op1=ALU.mult)
            tp = tps.tile([128, 256], FP32, name="tp_b", tag="tp")
            nc.tensor.transpose(tp[:, 0:128], hb[:, 0:128], ident2[:, :])
            nc.tensor.transpose(tp[:, 128:256], hb[:, 128:256], ident2[:, :])
            hbT = st.tile([128, 256], BF16, name="hbT")
            nc.scalar.copy(hbT[:, 0:128], tp[:, 0:128])
            nc.vector.tensor_copy(hbT[:, 128:256], tp[:, 128:256])

            # x (feature-major [256, 128]) = relu(tmpW.T @ [hT; hbT] + b)
            xps = tps.tile([128, 256], FP32, name="xps", tag="tp")
            for oc in range(2):
                for kc in range(4):
                    rhs = hT[:, kc * 128:kc * 128 + 128] if kc < 2 else \
                        hbT[:, (kc - 2) * 128:(kc - 2) * 128 + 128]
                    nc.tensor.matmul(
                        xps[:, oc * 128:(oc + 1) * 128],
                        lhsT=w["tmpW"][:, kc * 256 + oc * 128:
                                       kc * 256 + oc * 128 + 128],
                        rhs=rhs, start=(kc == 0), stop=(kc == 3))
            x_fm = st.tile([128, 256], BF16, name="x_fm")
            for oc in range(2):
                nc.scalar.activation(x_fm[:, oc * 128:(oc + 1) * 128],
                                     xps[:, oc * 128:(oc + 1) * 128],
                                     AF.Relu, bias=bcol("tmp", oc))

            # q_bm[b, out] = x @ Wx.T + b  (batch-major, bias via ones-row;
            # per-tile 8-row slices feed K=8 selector matmuls)
            q_bm = {}
            for qi, (qn, wn) in enumerate([("qwr", "wr1x"), ("qf", "fWx"),
                                           ("qi", "iWx"), ("qc", "cWx"),
                                           ("qrd", "rd1x")]):
                ps = tps.tile([128, 256], FP32, name=f"ps_{qn}", tag="tp")
                nc.tensor.matmul(ps[:, :], lhsT=ones1[:, :],
                                 rhs=qbrow[0:1, qi * 256:(qi + 1) * 256],
                                 start=True, stop=False)
                for kc in range(2):
                    nc.tensor.matmul(
                        ps[:, :], lhsT=x_fm[:, kc * 128:kc * 128 + 128],
                        rhs=w[wn][:, kc * 256:(kc + 1) * 256],
                        start=False, stop=(kc == 1))
                q = st.tile([128, 256], BF16, name=f"qbm_{qn}")
                nc.scalar.copy(q[:, :], ps[:, :])
                q_bm[qi] = q

            # ============================================================
            # Memory module: 16 tiles x 512 cols (8 samples each)
            # ============================================================
            for n in range(NT_C):
                cs = slice(n * NTILE, (n + 1) * NTILE)
                sub_t = subp.tile([128, 2, NTILE], BF16, name="sub_t",
                                  tag="sub")
                for kc in range(2):
                    nc.sync.dma_start(out=sub_t[:, kc],
                                      in_=d["sub"][kc * 128:(kc + 1) * 128, cs])
                qrow8 = cp.tile([8, 5, 256], BF16, name="qrow8", tag="qrow")
                for qi in range(5):
                    nc.sync.dma_start(out=qrow8[:, qi],
                                      in_=q_bm[qi][n * SPT:(n + 1) * SPT, :])

                def cat_linear(ws_name, qi, act, bufname):
                    outs = []
                    odt = BF16 if act == "relu" else FP32
                    for oc in range(2):
                        ps = efps.tile([128, NTILE], FP32, name=f"ps_{bufname}",
                                       tag="efps")
                        nc.tensor.matmul(
                            ps[:, :],
                            lhsT=qrow8[:, qi, oc * 128:oc * 128 + 128],
                            rhs=sel8[:, :], start=True, stop=False)
                        for kc in range(2):
                            nc.tensor.matmul(
                                ps[:, :],
                                lhsT=w[ws_name][:, kc * 256 + oc * 128:
                                                kc * 256 + oc * 128 + 128],
                                rhs=sub_t[:, kc],
                                start=False, stop=(kc == 1))
                        ot = cp.tile([128, NTILE], odt, name=f"{bufname}{oc}",
                                     tag=f"{bufname}{oc}")
                        if act == "relu":
                            if oc == 0:
                                nc.scalar.activation(ot[:, :], ps[:, :],
                                                     AF.Relu)
                            else:
                                nc.vector.tensor_scalar(
                                    out=ot[:, :], in0=ps[:, :],
                                    scalar1=0.0, scalar2=None, op0=ALU.max)
                        else:
                            nc.scalar.activation(ot[:, :], ps[:, :],
                                                 AF.Sigmoid)
                        outs.append(ot)
                    return outs

                r1 = cat_linear("wr1s", 0, "relu", "r1")
                zp = gps.tile([1, NTILE], FP32, name="zp", tag="g")
                for kc in range(2):
                    nc.tensor.matmul(zp[:, :], lhsT=w["w2pack"][:, kc:kc + 1],
                                     rhs=r1[kc][:, :],
                                     start=(kc == 0), stop=(kc == 1))
                ex1 = rowp.tile([1, NTILE], FP32, name="ex1", tag="ex1")
                exn = rowp.tile([1, NTILE], FP32, name="exn", tag="exn")
                nc.scalar.activation(ex1[:, :], zp[:, :], AF.Sigmoid)
                nc.scalar.activation(exn[:, :], zp[:, :], AF.Sigmoid,
                                     scale=-1.0)
                nc.vector.reciprocal(exn[:, :], exn[:, :])
                nc.vector.tensor_mul(ex1[:, :], ex1[:, :], exn[:, :])
                s1 = rowp.tile([1, SPT], FP32, name="s1", tag="s1")
                nc.vector.tensor_reduce(
                    s1[:, :], ex1[:, :].rearrange("p (b m) -> p b m", b=SPT),
                    axis=mybir.AxisListType.X, op=ALU.add)
                rc1 = rowp.tile([1, SPT], FP32, name="rc1", tag="rc1")
                nc.vector.reciprocal(rc1[:, :], s1[:, :])
                nc.vector.tensor_mul(
                    ex1[:, :].rearrange("p (b m) -> p b m", b=SPT),
                    ex1[:, :].rearrange("p (b m) -> p b m", b=SPT),
                    rc1[:, :].unsqueeze(2).broadcast_to([1, SPT, M]))
                simb_t = cp.tile([128, NTILE], FP32, name="simb_t",
                                 tag="bcast")
                nc.gpsimd.partition_broadcast(simb_t[:, :], ex1[:, :])
                simb = simb_t[:, :]

                f_t = cat_linear("fWs", 1, "sig", "ft")
                i_t = cat_linear("iWs", 2, "sig", "it")
                c_t = cat_linear("cWs", 3, "sig", "ct")   # holds s_c

                # mem_pre = sub - sim*(f*sub + i*(1 - 2*s_c))   (in place)
                for oc in range(2):
                    nc.gpsimd.tensor_scalar(
                        out=c_t[oc][:, :], in0=c_t[oc][:, :], scalar1=-2.0,
                        scalar2=1.0, op0=ALU.mult, op1=ALU.add)
                    nc.gpsimd.tensor_mul(c_t[oc][:, :], c_t[oc][:, :],
                                         i_t[oc][:, :])
                    nc.vector.tensor_mul(f_t[oc][:, :], f_t[oc][:, :],
                                         sub_t[:, oc])
                    nc.vector.tensor_add(f_t[oc][:, :], f_t[oc][:, :],
                                         c_t[oc][:, :])
                    nc.gpsimd.tensor_mul(f_t[oc][:, :], f_t[oc][:, :], simb)
                    nc.vector.tensor_sub(sub_t[:, oc], sub_t[:, oc],
                                         f_t[oc][:, :])

                # mem_s = sigmoid(2*(sd@mem_pre) + 2*sd_b); mem = 2*mem_s-1
                # (weights pre-doubled on host; rd1/pre folded downstream;
                #  fp32 mem_s DMA'd out, host applies 2s-1)
                mem_s, mem_b = [], []
                for oc in range(2):
                    ps = efps.tile([128, NTILE], FP32, name="ps_sd",
                                   tag="efps")
                    for kc in range(2):
                        nc.tensor.matmul(
                            ps[:, :],
                            lhsT=w["sdW"][:, kc * 256 + oc * 128:
                                          kc * 256 + oc * 128 + 128],
                            rhs=sub_t[:, kc],
                            start=(kc == 0), stop=(kc == 1))
                    ms = cp.tile([128, NTILE], FP32, name=f"mems{oc}",
                                 tag=f"mems{oc}")
                    nc.scalar.activation(ms[:, :], ps[:, :], AF.Sigmoid,
                                         bias=bcol("sd", oc))
                    nc.sync.dma_start(out=d["mout"][oc * 128:(oc + 1) * 128, cs],
                                      in_=ms[:, :])
                    mb = cp.tile([128, NTILE], BF16, name=f"memb{oc}",
                                 tag=f"memb{oc}")
                    nc.scalar.copy(mb[:, :], ms[:, :])
                    mem_s.append(ms)
                    mem_b.append(mb)

                # read attention (rd1s/q_rd host-folded for mem = 2s-1)
                r2 = []
                for oc in range(2):
                    ps = efps.tile([128, NTILE], FP32, name="ps_r2",
                                   tag="efps")
                    nc.tensor.matmul(
                        ps[:, :], lhsT=qrow8[:, 4, oc * 128:oc * 128 + 128],
                        rhs=sel8[:, :], start=True, stop=False)
                    for kc in range(2):
                        nc.tensor.matmul(
                            ps[:, :],
                            lhsT=w["rd1s"][:, kc * 256 + oc * 128:
                                           kc * 256 + oc * 128 + 128],
                            rhs=mem_b[kc][:, :],
                            start=False, stop=(kc == 1))
                    ot = cp.tile([128, NTILE], BF16, name=f"r2{oc}",
                                 tag=f"r1{oc}")
                    if oc == 0:
                        nc.scalar.activation(ot[:, :], ps[:, :], AF.Relu)
                    else:
                        nc.vector.tensor_scalar(
                            out=ot[:, :], in0=ps[:, :], scalar1=0.0,
                            scalar2=None, op0=ALU.max)
                    r2.append(ot)
                zp2 = gps.tile([1, NTILE], FP32, name="zp2", tag="g")
                for kc in range(2):
                    nc.tensor.matmul(zp2[:, :],
                                     lhsT=w["w2pack"][:, 2 + kc:3 + kc],
                                     rhs=r2[kc][:, :],
                                     start=(kc == 0), stop=(kc == 1))
                ex2 = rowp.tile([1, NTILE], FP32, name="ex2", tag="ex1")
                ex2n = rowp.tile([1, NTILE], FP32, name="ex2n", tag="exn")
                nc.scalar.activation(ex2[:, :], zp2[:, :], AF.Sigmoid)
                nc.scalar.activation(ex2n[:, :], zp2[:, :], AF.Sigmoid,
                                     scale=-1.0)
                nc.vector.reciprocal(ex2n[:, :], ex2n[:, :])
                nc.vector.tensor_mul(ex2[:, :], ex2[:, :], ex2n[:, :])
                nc.vector.tensor_reduce(
                    s2row[:, n * SPT:(n + 1) * SPT],
                    ex2[:, :].rearrange("p (b m) -> p b m", b=SPT),
                    axis=mybir.AxisListType.X, op=ALU.add)
                ex2b_t = cp.tile([128, NTILE], FP32, name="ex2b_t",
                                 tag="bcast")
                nc.gpsimd.partition_broadcast(ex2b_t[:, :], ex2[:, :])
                # P = sum_m s*e2; sub_vec = 2*P/S2 - 1 (affine at the tail)
                for oc in range(2):
                    wm = f_t[oc]
                    nc.gpsimd.tensor_mul(wm[:, :], mem_s[oc][:, :],
                                         ex2b_t[:, :])
                    nc.vector.tensor_reduce(
                        sv_un[:, oc * 128 + n * SPT: oc * 128 + (n + 1) * SPT],
                        wm[:, :].rearrange("p (b m) -> p b m", b=SPT),
                        axis=mybir.AxisListType.X, op=ALU.add)

            # ---- tail: sub_vec, final, y ------------------------------
            rc2 = rowp.tile([1, BC], FP32, name="rc2", tag="rc2")
            nc.vector.reciprocal(rc2[:, :], s2row[:, :])
            nc.vector.tensor_scalar(out=rc2[:, :], in0=rc2[:, :],
                                    scalar1=2.0, scalar2=None, op0=ALU.mult)
            rc2b_t = cp.tile([128, BC], FP32, name="rc2b_t", tag="bcast")
            nc.gpsimd.partition_broadcast(rc2b_t[:, :], rc2[:, :])
            sv_r = st.tile([128, 256], BF16, name="sv_r")
            for oc in range(2):
                nc.vector.tensor_mul(sv_un[:, oc * 128:(oc + 1) * 128],
                                     sv_un[:, oc * 128:(oc + 1) * 128],
                                     rc2b_t[:, :])
                nc.vector.tensor_scalar(
                    out=sv_r[:, oc * 128:(oc + 1) * 128],
                    in0=sv_un[:, oc * 128:(oc + 1) * 128],
                    scalar1=-1.0, scalar2=None, op0=ALU.add)
            fps_ = tps.tile([128, 256], FP32, name="fps", tag="tp")
            for oc in range(2):
                for kc in range(2):
                    nc.tensor.matmul(
                        fps_[:, oc * 128:(oc + 1) * 128],
                        lhsT=w["finW"][:, kc * 256 + oc * 128:
                                       kc * 256 + oc * 128 + 128],
                        rhs=sv_r[:, kc * 128:kc * 128 + 128],
                        start=(kc == 0), stop=(kc == 1))
            fin_t = st.tile([128, 256], BF16, name="fin_t")   # holds s_f
            for oc in range(2):
                nc.scalar.activation(fin_t[:, oc * 128:(oc + 1) * 128],
                                     fps_[:, oc * 128:(oc + 1) * 128],
                                     AF.Sigmoid, bias=bcol("fin", oc))
            yp = gps.tile([1, BC], FP32, name="yp", tag="g")
            for kc in range(2):
                nc.tensor.matmul(yp[:, :], lhsT=w["w2pack"][:, 4 + kc:5 + kc],
                                 rhs=fin_t[:, kc * 128:kc * 128 + 128],
                                 start=(kc == 0), stop=(kc == 1))
            y_t = rowp.tile([1, BC], FP32, name="y_t", tag="y_t")
            nc.scalar.activation(y_t[:, :], yp[:, :], AF.Sigmoid,
                                 bias=bp[0:1, BCOL["pre"]:BCOL["pre"] + 1])
            nc.sync.dma_start(out=d["yout"][:, :], in_=y_t[:, :])

    if not nc.is_finalized():
        nc.finalize()
    return nc


_NC_CACHE = None
LAST_RESULTS = None
LAST_IN_MAPS = None


def _get_nc():
    global _NC_CACHE
    if _NC_CACHE is None:
        _NC_CACHE = build_nc()
    return _NC_CACHE


def _prep_weights(p):
    g = {k: np.asarray(v, np.float32) for k, v in p.items()}
    out = {}
    out["vggW"] = _pack(g["vgg_W"].T)
    out["emoW"] = _pack(g["emo_W"].T)
    scn = np.zeros((384, 256), np.float32)
    scn[:365] = g["scn_W"].T
    out["scnW"] = _pack(scn)
    out["posW"] = _pack(g["pos_W"].T)
    out["fusW"] = _pack(g["fus_W"].T)

    lf_cols, lf_b = _gate_reorder(
        np.vstack([g["lf_Wih"].T, g["lf_Whh"].T]), g["lf_b"])
    out["lfW"] = _pack(lf_cols)
    lb_cols, lb_b = _gate_reorder(g["lb_Wih"].T, g["lb_b"])
    out["lbW"] = _pack(lb_cols)
    out["brow"] = np.concatenate([lf_b, lb_b]).reshape(1, 2048)
    out["tmpW"] = _pack(g["tmp_W"].T)

    out["wr1s"] = _pack(g["wr1_W"][:, :256].T)
    out["wr1x"] = _pack(g["wr1_W"][:, 256:].T)
    out["fWs"] = _pack(g["f_W"][:, :256].T)
    out["fWx"] = _pack(g["f_W"][:, 256:].T)
    out["iWs"] = _pack(g["i_W"][:, :256].T)
    out["iWx"] = _pack(g["i_W"][:, 256:].T)
    # c gate: tanh -> sigmoid(2x)
    out["cWs"] = _pack(2.0 * g["c_W"][:, :256].T)
    out["cWx"] = _pack(2.0 * g["c_W"][:, 256:].T)
    # rd1: mem = 2s-1 folded: W' = 2W, b' = b - rowsum(W_mem_part)
    rd1s_o = g["rd1_W"][:, :256]
    out["rd1s"] = _pack(2.0 * rd1s_o.T)
    out["rd1x"] = _pack(g["rd1_W"][:, 256:].T)
    rd1_b_adj = g["rd1_b"] - rd1s_o.sum(axis=1)
    # sd / fin: tanh -> sigmoid(2x)
    out["sdW"] = _pack(2.0 * g["sd_W"].T)
    out["finW"] = _pack(2.0 * g["fin_W"].T)

    # pre: y = sig(pre@(2s-1)+b) = sig((2 pre)@s + b - sum(pre))
    pre_w = 2.0 * g["pre_W"].reshape(256)
    pre_b_adj = float(np.asarray(g["pre_b"]).reshape(-1)[0]) - float(
        g["pre_W"].reshape(256).sum())

    w2 = np.zeros((128, 6), np.float32)
    w2[:, 0:2] = g["wr2_W"].reshape(256).reshape(2, 128).T
    w2[:, 2:4] = g["rd2_W"].reshape(256).reshape(2, 128).T
    w2[:, 4:6] = pre_w.reshape(2, 128).T
    out["w2pack"] = w2

    sel = np.zeros((8, 512), np.float32)
    for b_ in range(8):
        sel[b_, b_ * 64:(b_ + 1) * 64] = 1.0
    out["sel8"] = sel
    out["qbrow"] = np.concatenate(
        [g["wr1_b"], g["f_b"], g["i_b"], 2.0 * g["c_b"],
         rd1_b_adj]).reshape(1, 1280)

    bpk = np.zeros((128, NBCOL), np.float32)
    for nm, vec in [("vgg", g["vgg_b"]), ("emo", g["emo_b"]),
                    ("scn", g["scn_b"]), ("pos", g["pos_b"]),
                    ("fus", g["fus_b"]), ("tmp", g["tmp_b"]),
                    ("wr1", g["wr1_b"]), ("f", g["f_b"]), ("i", g["i_b"]),
                    ("c", 2.0 * g["c_b"]), ("rd1", rd1_b_adj),
                    ("sd", 2.0 * g["sd_b"]), ("fin", 2.0 * g["fin_b"])]:
        bpk[:, BCOL[nm]:BCOL[nm] + 2] = np.asarray(vec).reshape(2, 128).T
    bpk[0, BCOL["pre"]] = pre_b_adj

    for k in out:
        out[k] = out[k].astype(BFNP)
    out["bpack"] = bpk
    return out


def kernel(vggish, bgm_emotion, emotion, scene, pose, sub_memory, target,
           params, epochs, training, **_unused):
    vggish = np.asarray(vggish, np.float32)
    emotion = np.asarray(emotion, np.float32)
    scene = np.asarray(scene, np.float32)
    pose = np.asarray(pose, np.float32)
    sub_memory = np.asarray(sub_memory, np.float32)

    wmaps = _prep_weights(params)
    nc = _get_nc()

    in_maps = []
    for ci in range(NCORES):
        bs = slice(ci * BC, (ci + 1) * BC)
        xin = np.empty((1536, COLS), BFNP)
        xin[0:128] = vggish[bs].transpose(2, 1, 0).reshape(128, COLS)
        xin[128:640] = emotion[bs].transpose(2, 1, 0).reshape(512, COLS)
        xin[640:1005] = scene[bs].transpose(2, 1, 0).reshape(365, COLS)
        xin[1005:1024] = 0.0
        xin[1024:1536] = pose[bs].transpose(2, 1, 0).reshape(512, COLS)
        sub = np.ascontiguousarray(
            sub_memory[bs].transpose(2, 0, 1).reshape(256, MCOLS)).astype(BFNP)
        m = {"xin": xin, "sub": sub, "ones": np.ones((1, 128), BFNP)}
        m.update(wmaps)
        in_maps.append(m)

    global LAST_RESULTS, LAST_IN_MAPS
    LAST_IN_MAPS = in_maps
    res = run_bass_kernel_spmd(nc, in_maps, list(range(NCORES)))
    LAST_RESULTS = res

    mem = np.empty((B, M, F), np.float32)
    y = np.empty((B, 1), np.float32)
    for ci in range(NCORES):
        bs = slice(ci * BC, (ci + 1) * BC)
        mo = res.results[ci]["mout"]            # fp32 sigmoid values
        mem[bs] = 2.0 * mo.reshape(256, BC, M).transpose(1, 2, 0) - 1.0
        y[bs] = res.results[ci]["yout"].reshape(BC, 1)
    return mem, y


if __name__ == "__main__":
    print("building nc...")
    nc = _get_nc()
    print("ok")


# revision 23
# speedup vs baseline: 2.4588x; 2.0629x over previous
"""Trainium2 Bass kernel for nn_Actor (scatter_memory).

Pure data parallel across 8 NeuronCores (128 samples each, no
collectives). Activations are feature-major ([feat_part, cols]); every
linear is lhsT=W.T chunks / rhs=activation chunks in bf16 (fp32 PSUM
accumulate). The LSTM uses batch-major gates (lhsT = x_t/h tiles,
rhs = weight rows); h returns to feature-major via PE transpose each
step. Memory-module columns are b-major (col = b*64 + m) so softmax
over M is local to each 512-column tile.

All ACT transcendentals use the sigmoid table: tanh(x) = 2*sigmoid(2x)-1
with the 2x folded into host-scaled weights and the affine folded into
neighbouring DVE ops or downstream weights (rd1/pre) or the host-side
output transform (mem = 2*s - 1).
"""

import os
import sys

import numpy as np
import ml_dtypes

for _p in ("/opt/trn_rl_repo",):
    if _p not in sys.path:
        sys.path.insert(0, _p)

os.environ.setdefault("JAX_PLATFORMS", "")

import concourse.bass as bass
import concourse.bacc as bacc
import concourse.mybir as mybir
import concourse.tile as tile
from concourse.bass_utils import run_bass_kernel_spmd
from concourse.masks import make_identity

FP32 = mybir.dt.float32
BF16 = mybir.dt.bfloat16
AF = mybir.ActivationFunctionType
ALU = mybir.AluOpType
BFNP = ml_dtypes.bfloat16

B, T, M, F = 1024, 64, 64, 256
NCORES = 8
BC = B // NCORES          # 128 samples per core
COLS = T * BC             # phase-A columns (t-major: col = t*128 + b)
MCOLS = BC * M            # memory columns (b-major: col = b*64 + m)
NTILE = 512
NT_A = COLS // NTILE      # 16
NT_C = MCOLS // NTILE     # 16
SPT = 8                   # samples per 512-col memory tile

EF_BLOCKS = [
    ("v", 0, 1),    # vggish   rows 0:128
    ("e", 1, 4),    # emotion  rows 128:640
    ("s", 5, 3),    # scene    rows 640:1024 (365 padded to 384)
    ("p", 8, 4),    # pose     rows 1024:1536
]

BCOL = {}
_c = 0
for _n in ("vgg", "emo", "scn", "pos", "fus", "tmp", "wr1", "f", "i", "c",
           "rd1", "sd", "fin"):
    BCOL[_n] = _c
    _c += 2
BCOL["pre"] = _c
NBCOL = _c + 1


def _pack(wt):
    """[kin, out] (kin % 128 == 0) -> [128, (kin/128)*out]."""
    kin, out = wt.shape
    assert kin % 128 == 0
    return np.ascontiguousarray(
        wt.reshape(kin // 128, 128, out).transpose(1, 0, 2).reshape(128, -1))


def _gate_reorder(w_cols, b):
    """[i f g o] -> [i f o g]; double the g block (tanh-as-sigmoid)."""
    i, f, g, o = (w_cols[:, k * F:(k + 1) * F] for k in range(4))
    w = np.concatenate([i, 2.0 * g, f, o], axis=1)
    bi, bf, bg, bo = (b[k * F:(k + 1) * F] for k in range(4))
    return w, np.concatenate([bi, 2.0 * bg, bf, bo])


def build_nc():
    nc = bacc.Bacc(None)
    d = {}

    def param(name, shape, out=False, dt=FP32):
        d[name] = nc.declare_dram_parameter(name, list(shape), dt, isOutput=out)
        return d[name]

    param("xin", (1536, COLS), dt=BF16)
    param("sub", (F, MCOLS), dt=BF16)
    WSPEC = [("vggW", 256), ("emoW", 1024), ("scnW", 768), ("posW", 1024),
             ("fusW", 2048), ("lfW", 4096), ("lbW", 2048), ("tmpW", 1024),
             ("wr1s", 512), ("wr1x", 512), ("fWs", 512), ("fWx", 512),
             ("iWs", 512), ("iWx", 512), ("cWs", 512), ("cWx", 512),
             ("rd1s", 512), ("rd1x", 512), ("sdW", 512), ("finW", 512),
             ("w2pack", 6)]
    for n, c in WSPEC:
        param(n, (128, c), dt=BF16)
    param("bpack", (128, NBCOL))
    param("sel8", (8, NTILE), dt=BF16)
    param("qbrow", (1, 1280), dt=BF16)
    param("brow", (1, 2048), dt=BF16)
    param("ones", (1, 128), dt=BF16)
    param("mout", (F, MCOLS), out=True)
    param("yout", (1, BC), out=True)

    with tile.TileContext(nc) as tc:
        with (
            tc.tile_pool(name="wp", bufs=1) as wp,
            tc.tile_pool(name="state", bufs=1) as st,
            tc.tile_pool(name="efin", bufs=2) as efin,
            tc.tile_pool(name="actp", bufs=2) as actp,
            tc.tile_pool(name="fusedp", bufs=3) as fusedp,
            tc.tile_pool(name="lsp", bufs=2) as lsp,
            tc.tile_pool(name="subp", bufs=3) as subp,
            tc.tile_pool(name="cp", bufs=3) as cp,
            tc.tile_pool(name="rowp", bufs=2) as rowp,
            tc.tile_pool(name="efps", bufs=3, space="PSUM") as efps,
            tc.tile_pool(name="gps", bufs=2, space="PSUM") as gps,
            tc.tile_pool(name="tps", bufs=1, space="PSUM") as tps,
        ):
            # ---- weights ----------------------------------------------
            w = {}
            for n, c in WSPEC:
                w[n] = wp.tile([128, c], BF16, name=f"w_{n}")
                nc.sync.dma_start(out=w[n][:, :], in_=d[n][:, :])
            w["bpack"] = wp.tile([128, NBCOL], FP32, name="w_bpack")
            nc.sync.dma_start(out=w["bpack"][:, :], in_=d["bpack"][:, :])
            brow = wp.tile([1, 2048], BF16, name="w_brow")
            nc.sync.dma_start(out=brow[:, :], in_=d["brow"][:, :])
            sel8 = wp.tile([8, NTILE], BF16, name="w_sel8")
            nc.sync.dma_start(out=sel8[:, :], in_=d["sel8"][:, :])
            qbrow = wp.tile([1, 1280], BF16, name="w_qbrow")
            nc.sync.dma_start(out=qbrow[:, :], in_=d["qbrow"][:, :])
            ones1 = wp.tile([1, 128], BF16, name="ones1")
            nc.sync.dma_start(out=ones1[:, :], in_=d["ones"][:, :])
            ident = wp.tile([128, 128], FP32, name="ident")
            make_identity(nc, ident[:, :])
            ident2 = wp.tile([128, 128], FP32, name="ident2")
            make_identity(nc, ident2[:, :])
            nc.scalar.mul(ident2[:, :], ident2[:, :], 2.0)
            bp = w["bpack"]

            def bcol(name, oc=0):
                c0 = BCOL[name] + oc
                return bp[:, c0:c0 + 1]

            # ---- persistent state -------------------------------------
            hT = st.tile([128, 256], BF16, name="hT")
            c_bm = st.tile([128, 256], FP32, name="c_bm")
            nc.vector.memset(hT[:, :], 0.0)
            nc.vector.memset(c_bm[:, :], 0.0)
            sv_un = st.tile([128, 256], FP32, name="sv_un")
            s2row = st.tile([1, BC], FP32, name="s2row")

            # ============================================================
            # Phase A (early fusion) emitted in quarter-tile slices
            # between LSTM steps (fills PE during the step's serial
            # elementwise chain; the PE queue is in-order, so ready
            # matmuls must be emitted ahead of blocking step matmuls).
            # ============================================================
            WNAME = {"v": "vggW", "e": "emoW", "s": "scnW", "p": "posW"}
            BNAME = {"v": "vgg", "e": "emo", "s": "scn", "p": "pos"}

            def ef_input(n, stt_, name, blk0, nk, ei):
                xin_t = stt_["xin"]
                for oc in range(2):
                    ps = efps.tile([128, NTILE], FP32, name="ps_ef",
                                   tag="efps")
                    wt = w[WNAME[name]]
                    for kc in range(nk):
                        nc.tensor.matmul(
                            ps[:, :],
                            lhsT=wt[:, kc * 256 + oc * 128:
                                    kc * 256 + oc * 128 + 128],
                            rhs=xin_t[:, blk0 + kc],
                            start=(kc == 0), stop=(kc == nk - 1))
                    at = actp.tile([128, NTILE], BF16, name=f"act_{name}{oc}",
                                   tag=f"act_{name}{oc}")
                    bc = bcol(BNAME[name], oc)
                    if (ei + oc) % 2 == 0:
                        nc.scalar.activation(at[:, :], ps[:, :], AF.Relu,
                                             bias=bc)
                    else:
                        nc.vector.tensor_scalar(
                            out=at[:, :], in0=ps[:, :], scalar1=bc,
                            scalar2=0.0, op0=ALU.add, op1=ALU.max)
                    stt_["acts"][(name, oc)] = at

            def ef_quarter(n, q, stt_):
                cs = slice(n * NTILE, (n + 1) * NTILE)
                if q == 0:
                    xin_t = efin.tile([128, 12, NTILE], BF16, name="xin_t")
                    stt_["xin"] = xin_t
                    stt_["acts"] = {}
                    for j in range(12):
                        nc.sync.dma_start(
                            out=xin_t[:, j],
                            in_=d["xin"][j * 128:(j + 1) * 128, cs])
                    ef_input(n, stt_, "v", 0, 1, 0)
                elif q == 1:
                    ef_input(n, stt_, "e", 1, 4, 1)
                elif q == 2:
                    ef_input(n, stt_, "s", 5, 3, 0)
                    ef_input(n, stt_, "p", 8, 4, 1)
                else:
                    acts = stt_["acts"]
                    fused = []
                    for oc in range(2):
                        ps = efps.tile([128, NTILE], FP32, name="ps_fus",
                                       tag="efps")
                        mi = 0
                        for ai, (name, _, _) in enumerate(EF_BLOCKS):
                            for kc in range(2):
                                nc.tensor.matmul(
                                    ps[:, :],
                                    lhsT=w["fusW"][:, (ai * 2 + kc) * 256 + oc * 128:
                                                   (ai * 2 + kc) * 256 + oc * 128 + 128],
                                    rhs=acts[(name, kc)][:, :],
                                    start=(mi == 0), stop=(mi == 7))
                                mi += 1
                        ft = fusedp.tile([128, NTILE], BF16,
                                         name=f"fused{oc}", tag=f"fused{oc}")
                        if oc == 0:
                            nc.scalar.activation(ft[:, :], ps[:, :], AF.Relu,
                                                 bias=bcol("fus", oc))
                        else:
                            nc.vector.tensor_scalar(
                                out=ft[:, :], in0=ps[:, :],
                                scalar1=bcol("fus", oc), scalar2=0.0,
                                op0=ALU.add, op1=ALU.max)
                        fused.append(ft)
                    stt_["fused"] = fused

            def lstm_step(tt, fused):
                # gates (batch-major, cols reordered [i g' f o], g' = 2g):
                # s = sigmoid(gp); g~ = 2*s_g - 1
                # c = f*c + 2*(i*s_g) - i ;  h = 2*(o*sigmoid(2c)) - o
                t4 = tt * 128
                gp = gps.tile([128, 1024], FP32, name="gp", tag="g")
                for nh in range(2):
                    out_ap = gp[:, nh * 512:(nh + 1) * 512]
                    nc.tensor.matmul(out_ap, lhsT=ones1[:, :],
                                     rhs=brow[0:1, nh * 512:(nh + 1) * 512],
                                     start=True, stop=False)
                    for kc in range(4):
                        if kc < 2:
                            lhs = fused[kc][:, t4:t4 + 128]
                        else:
                            lhs = hT[:, (kc - 2) * 128:(kc - 1) * 128]
                        nc.tensor.matmul(
                            out_ap, lhsT=lhs,
                            rhs=w["lfW"][:, kc * 1024 + nh * 512:
                                         kc * 1024 + nh * 512 + 512],
                            start=False, stop=(kc == 3))
                gall = lsp.tile([128, 1024], FP32, name="gall", tag="gall")
                nc.scalar.activation(gall[:, 0:512], gp[:, 0:512], AF.Sigmoid)
                nc.scalar.activation(gall[:, 512:1024], gp[:, 512:1024],
                                     AF.Sigmoid)
                gi, gs = gall[:, 0:256], gall[:, 256:512]
                gf, go = gall[:, 512:768], gall[:, 768:1024]
                # t1 = i*(s_g - 0.5); c = f*c + 2*t1
                t1 = lsp.tile([128, 256], FP32, name="t1", tag="t1")
                nc.vector.scalar_tensor_tensor(
                    out=t1[:, :], in0=gs, scalar=-0.5, in1=gi,
                    op0=ALU.add, op1=ALU.mult)
                nc.vector.tensor_mul(c_bm[:, :], c_bm[:, :], gf)
                nc.vector.scalar_tensor_tensor(
                    out=c_bm[:, :], in0=t1[:, :], scalar=2.0,
                    in1=c_bm[:, :], op0=ALU.mult, op1=ALU.add)
                sc = lsp.tile([128, 256], FP32, name="sc", tag="sc")
                nc.scalar.activation(sc[:, :], c_bm[:, :], AF.Sigmoid,
                                     scale=2.0)
                # h = 2*o*(s_c-0.5); the 2x rides the transpose (ident2)
                h_bm = lsp.tile([128, 256], FP32, name="h_bm", tag="h_bm")
                nc.vector.scalar_tensor_tensor(
                    out=h_bm[:, :], in0=sc[:, :], scalar=-0.5, in1=go,
                    op0=ALU.add, op1=ALU.mult)
                tp = tps.tile([128, 256], FP32, name="tp", tag="tp")
                nc.tensor.transpose(tp[:, 0:128], h_bm[:, 0:128], ident2[:, :])
                nc.tensor.transpose(tp[:, 128:256], h_bm[:, 128:256],
                                    ident2[:, :])
                nc.scalar.copy(hT[:, 0:128], tp[:, 0:128])
                nc.vector.tensor_copy(hT[:, 128:256], tp[:, 128:256])

            st_cur = {}
            for q in range(4):
                ef_quarter(0, q, st_cur)
            fused_cur = st_cur["fused"]
            st_nxt = {}
            for t in range(T):
                lstm_step(t % 4, fused_cur)
                n_next = t // 4 + 1
                if n_next < NT_A:
                    ef_quarter(n_next, t % 4, st_nxt)
                    if t % 4 == 3:
                        fused_cur = st_nxt["fused"]
                        st_nxt = {}
            last_fused = fused_cur

            # ============================================================
            # backward cell (t = T-1), then x = relu(tmp([h_fwd; h_bwd]))
            # ============================================================
            gp = gps.tile([128, 1024], FP32, name="gp_b", tag="g")
            for nh in range(2):
                out_ap = gp[:, nh * 512:(nh + 1) * 512]
                nc.tensor.matmul(out_ap, lhsT=ones1[:, :],
                                 rhs=brow[0:1, 1024 + nh * 512:
                                          1024 + (nh + 1) * 512],
                                 start=True, stop=False)
                for kc in range(2):
                    nc.tensor.matmul(
                        out_ap, lhsT=last_fused[kc][:, 384:512],
                        rhs=w["lbW"][:, kc * 1024 + nh * 512:
                                     kc * 1024 + nh * 512 + 512],
                        start=False, stop=(kc == 1))
            gall = lsp.tile([128, 1024], FP32, name="gall_b", tag="gall")
            nc.scalar.activation(gall[:, :], gp[:, :], AF.Sigmoid)
            gi, gs, go = gall[:, 0:256], gall[:, 256:512], gall[:, 768:1024]
            t1 = lsp.tile([128, 256], FP32, name="t1b", tag="t1")
            nc.vector.scalar_tensor_tensor(
                out=t1[:, :], in0=gs, scalar=-0.5, in1=gi,
                op0=ALU.add, op1=ALU.mult)
            scb = lsp.tile([128, 256], FP32, name="scb", tag="t2")
            nc.scalar.activation(scb[:, :], t1[:, :], AF.Sigmoid, scale=4.0)
            hb = lsp.tile([128, 256], FP32, name="hb", tag="h_bm")
            nc.vector.scalar_tensor_tensor(
                out=hb[:, :], in0=scb[:, :], scalar=-0.5, in1=go,
                op0=ALU.add, op1=ALU.mult)
            tp = tps.tile([128, 256], FP32, name="tp_b", tag="tp")
            nc.tensor.transpose(tp[:, 0:128], hb[:, 0:128], ident2[:, :])
            nc.tensor.transpose(tp[:, 128:256], hb[:, 128:256], ident2[:, :])
            hbT = st.tile([128, 256], BF16, name="hbT")
            nc.scalar.copy(hbT[:, 0:128], tp[:, 0:128])
            nc.vector.tensor_copy(hbT[:, 128:256], tp[:, 128:256])

            # x (feature-major [256, 128]) = relu(tmpW.T @ [hT; hbT] + b)
            xps = tps.tile([128, 256], FP32, name="xps", tag="tp")
            for oc in range(2):
                for kc in range(4):
                    rhs = hT[:, kc * 128:kc * 128 + 128] if kc < 2 else \
                        hbT[:, (kc - 2) * 128:(kc - 2) * 128 + 128]
                    nc.tensor.matmul(
                        xps[:, oc * 128:(oc + 1) * 128],
                        lhsT=w["tmpW"][:, kc * 256 + oc * 128:
                                       kc * 256 + oc * 128 + 128],
                        rhs=rhs, start=(kc == 0), stop=(kc == 3))
            x_fm = st.tile([128, 256], BF16, name="x_fm")
            for oc in range(2):
                nc.scalar.activation(x_fm[:, oc * 128:(oc + 1) * 128],
                                     xps[:, oc * 128:(oc + 1) * 128],
                                     AF.Relu, bias=bcol("tmp", oc))

            # q_bm[b, out] = x @ Wx.T + b  (batch-major, bias via ones-row;
            # per-tile 8-row slices feed K=8 selector matmuls)
            q_bm = {}
            for qi, (qn, wn) in enumerate([("qwr", "wr1x"), ("qf", "fWx"),
                                           ("qi", "iWx"), ("qc", "cWx"),
                                           ("qrd", "rd1x")]):
                ps = tps.tile([128, 256], FP32, name=f"ps_{qn}", tag="tp")
                nc.tensor.matmul(ps[:, :], lhsT=ones1[:, :],
                                 rhs=qbrow[0:1, qi * 256:(qi + 1) * 256],
                                 start=True, stop=False)
                for kc in range(2):
                    nc.tensor.matmul(
                        ps[:, :], lhsT=x_fm[:, kc * 128:kc * 128 + 128],
                        rhs=w[wn][:, kc * 256:(kc + 1) * 256],
                        start=False, stop=(kc == 1))
                q = st.tile([128, 256], BF16, name=f"qbm_{qn}")
                nc.scalar.copy(q[:, :], ps[:, :])
                q_bm[qi] = q

            # ============================================================
            # Memory module: 16 tiles x 512 cols (8 samples each)
            # ============================================================
            for n in range(NT_C):
                cs = slice(n * NTILE, (n + 1) * NTILE)
                sub_t = subp.tile([128, 2, NTILE], BF16, name="sub_t",
                                  tag="sub")
                for kc in range(2):
                    nc.sync.dma_start(out=sub_t[:, kc],
                                      in_=d["sub"][kc * 128:(kc + 1) * 128, cs])
                qrow8 = cp.tile([8, 5, 256], BF16, name="qrow8", tag="qrow")
                for qi in range(5):
                    nc.sync.dma_start(out=qrow8[:, qi],
                                      in_=q_bm[qi][n * SPT:(n + 1) * SPT, :])

                def cat_linear(ws_name, qi, act, bufname):
                    outs = []
                    odt = BF16 if act == "relu" else FP32
                    for oc in range(2):
                        ps = efps.tile([128, NTILE], FP32, name=f"ps_{bufname}",
                                       tag="efps")
                        for kc in range(2):
                            nc.tensor.matmul(
                                ps[:, :],
                                lhsT=w[ws_name][:, kc * 256 + oc * 128:
                                                kc * 256 + oc * 128 + 128],
                                rhs=sub_t[:, kc],
                                start=(kc == 0), stop=False)
                        nc.tensor.matmul(
                            ps[:, :],
                            lhsT=qrow8[:, qi, oc * 128:oc * 128 + 128],
                            rhs=sel8[:, :], start=False, stop=True)
                        ot = cp.tile([128, NTILE], odt, name=f"{bufname}{oc}",
                                     tag=f"{bufname}{oc}")
                        if act == "relu":
                            if oc == 0:
                                nc.scalar.activation(ot[:, :], ps[:, :],
                                                     AF.Relu)
                            else:
                                nc.vector.tensor_scalar(
                                    out=ot[:, :], in0=ps[:, :],
                                    scalar1=0.0, scalar2=None, op0=ALU.max)
                        else:
                            nc.scalar.activation(ot[:, :], ps[:, :],
                                                 AF.Sigmoid)
                        outs.append(ot)
                    return outs

                r1 = cat_linear("wr1s", 0, "relu", "r1")
                zp = gps.tile([1, NTILE], FP32, name="zp", tag="g")
                for kc in range(2):
                    nc.tensor.matmul(zp[:, :], lhsT=w["w2pack"][:, kc:kc + 1],
                                     rhs=r1[kc][:, :],
                                     start=(kc == 0), stop=(kc == 1))
                ex1 = rowp.tile([1, NTILE], FP32, name="ex1", tag="ex1")
                exn = rowp.tile([1, NTILE], FP32, name="exn", tag="exn")
                nc.scalar.activation(ex1[:, :], zp[:, :], AF.Sigmoid)
                nc.scalar.activation(exn[:, :], zp[:, :], AF.Sigmoid,
                                     scale=-1.0)
                nc.vector.reciprocal(exn[:, :], exn[:, :])
                nc.vector.tensor_mul(ex1[:, :], ex1[:, :], exn[:, :])
                s1 = rowp.tile([1, SPT], FP32, name="s1", tag="s1")
                nc.vector.tensor_reduce(
                    s1[:, :], ex1[:, :].rearrange("p (b m) -> p b m", b=SPT),
                    axis=mybir.AxisListType.X, op=ALU.add)
                rc1 = rowp.tile([1, SPT], FP32, name="rc1", tag="rc1")
                nc.vector.reciprocal(rc1[:, :], s1[:, :])
                nc.vector.tensor_mul(
                    ex1[:, :].rearrange("p (b m) -> p b m", b=SPT),
                    ex1[:, :].rearrange("p (b m) -> p b m", b=SPT),
                    rc1[:, :].unsqueeze(2).broadcast_to([1, SPT, M]))
                simb_t = cp.tile([128, NTILE], FP32, name="simb_t",
                                 tag="bcast")
                nc.gpsimd.partition_broadcast(simb_t[:, :], ex1[:, :])
                simb = simb_t[:, :]

                f_t = cat_linear("fWs", 1, "sig", "ft")
                i_t = cat_linear("iWs", 2, "sig", "it")
                c_t = cat_linear("cWs", 3, "sig", "ct")   # holds s_c

                # mem_pre = sub - sim*(f*sub + i*(1 - 2*s_c))   (in place)
                for oc in range(2):
                    nc.gpsimd.tensor_scalar(
                        out=c_t[oc][:, :], in0=c_t[oc][:, :], scalar1=-2.0,
                        scalar2=1.0, op0=ALU.mult, op1=ALU.add)
                    nc.gpsimd.tensor_mul(c_t[oc][:, :], c_t[oc][:, :],
                                         i_t[oc][:, :])
                    nc.vector.tensor_mul(f_t[oc][:, :], f_t[oc][:, :],
                                         sub_t[:, oc])
                    nc.vector.tensor_add(f_t[oc][:, :], f_t[oc][:, :],
                                         c_t[oc][:, :])
                    nc.gpsimd.tensor_mul(f_t[oc][:, :], f_t[oc][:, :], simb)
                    nc.vector.tensor_sub(sub_t[:, oc], sub_t[:, oc],
                                         f_t[oc][:, :])

                # mem_s = sigmoid(2*(sd@mem_pre) + 2*sd_b); mem = 2*mem_s-1
                # (weights pre-doubled on host; rd1/pre folded downstream;
                #  fp32 mem_s DMA'd out, host applies 2s-1)
                mem_s = []
                for oc in range(2):
                    ps = efps.tile([128, NTILE], FP32, name="ps_sd",
                                   tag="efps")
                    for kc in range(2):
                        nc.tensor.matmul(
                            ps[:, :],
                            lhsT=w["sdW"][:, kc * 256 + oc * 128:
                                          kc * 256 + oc * 128 + 128],
                            rhs=sub_t[:, kc],
                            start=(kc == 0), stop=(kc == 1))
                    ms = cp.tile([128, NTILE], BF16, name=f"mems{oc}",
                                 tag=f"mems{oc}")
                    nc.scalar.activation(ms[:, :], ps[:, :], AF.Sigmoid,
                                         bias=bcol("sd", oc))
                    nc.gpsimd.dma_start(
                        out=d["mout"][oc * 128:(oc + 1) * 128, cs],
                        in_=ms[:, :])
                    mem_s.append(ms)
                mem_b = mem_s

                # read attention (rd1s/q_rd host-folded for mem = 2s-1)
                r2 = []
                for oc in range(2):
                    ps = efps.tile([128, NTILE], FP32, name="ps_r2",
                                   tag="efps")
                    for kc in range(2):
                        nc.tensor.matmul(
                            ps[:, :],
                            lhsT=w["rd1s"][:, kc * 256 + oc * 128:
                                           kc * 256 + oc * 128 + 128],
                            rhs=mem_b[kc][:, :],
                            start=(kc == 0), stop=False)
                    nc.tensor.matmul(
                        ps[:, :], lhsT=qrow8[:, 4, oc * 128:oc * 128 + 128],
                        rhs=sel8[:, :], start=False, stop=True)
                    ot = cp.tile([128, NTILE], BF16, name=f"r2{oc}",
                                 tag=f"r1{oc}")
                    if oc == 0:
                        nc.scalar.activation(ot[:, :], ps[:, :], AF.Relu)
                    else:
                        nc.vector.tensor_scalar(
                            out=ot[:, :], in0=ps[:, :], scalar1=0.0,
                            scalar2=None, op0=ALU.max)
                    r2.append(ot)
                zp2 = gps.tile([1, NTILE], FP32, name="zp2", tag="g")
                for kc in range(2):
                    nc.tensor.matmul(zp2[:, :],
                                     lhsT=w["w2pack"][:, 2 + kc:3 + kc],
                                     rhs=r2[kc][:, :],
                                     start=(kc == 0), stop=(kc == 1))
                ex2 = rowp.tile([1, NTILE], FP32, name="ex2", tag="ex1")
                ex2n = rowp.tile([1, NTILE], FP32, name="ex2n", tag="exn")
                nc.scalar.activation(ex2[:, :], zp2[:, :], AF.Sigmoid)
                nc.scalar.activation(ex2n[:, :], zp2[:, :], AF.Sigmoid,
                                     scale=-1.0)
                nc.vector.reciprocal(ex2n[:, :], ex2n[:, :])
                nc.vector.tensor_mul(ex2[:, :], ex2[:, :], ex2n[:, :])
                nc.vector.tensor_reduce(
                    s2row[:, n * SPT:(n + 1) * SPT],
                    ex2[:, :].rearrange("p (b m) -> p b m", b=SPT),
                    axis=mybir.AxisListType.X, op=ALU.add)
                ex2b_t = cp.tile([128, NTILE], FP32, name="ex2b_t",
                                 tag="bcast")
                nc.gpsimd.partition_broadcast(ex2b_t[:, :], ex2[:, :])
                # P = sum_m s*e2; sub_vec = 2*P/S2 - 1 (affine at the tail)
                for oc in range(2):
                    wm = f_t[oc]
                    nc.gpsimd.tensor_mul(wm[:, :], mem_s[oc][:, :],
                                         ex2b_t[:, :])
                    nc.vector.tensor_reduce(
                        sv_un[:, oc * 128 + n * SPT: oc * 128 + (n + 1) * SPT],
                        wm[:, :].rearrange("p (b m) -> p b m", b=SPT),
                        axis=mybir.AxisListType.X, op=ALU.add)

            # ---- tail: sub_vec, final, y ------------------------------
            rc2 = rowp.tile([1, BC], FP32, name="rc2", tag="rc2")
            nc.vector.reciprocal(rc2[:, :], s2row[:, :])
            nc.vector.tensor_scalar(out=rc2[:, :], in0=rc2[:, :],
                                    scalar1=2.0, scalar2=None, op0=ALU.mult)
            rc2b_t = cp.tile([128, BC], FP32, name="rc2b_t", tag="bcast")
            nc.gpsimd.partition_broadcast(rc2b_t[:, :], rc2[:, :])
            sv_r = st.tile([128, 256], BF16, name="sv_r")
            for oc in range(2):
                nc.vector.tensor_mul(sv_un[:, oc * 128:(oc + 1) * 128],
                                     sv_un[:, oc * 128:(oc + 1) * 128],
                                     rc2b_t[:, :])
                nc.vector.tensor_scalar(
                    out=sv_r[:, oc * 128:(oc + 1) * 128],
                    in0=sv_un[:, oc * 128:(oc + 1) * 128],
                    scalar1=-1.0, scalar2=None, op0=ALU.add)
            fps_ = tps.tile([128, 256], FP32, name="fps", tag="tp")
            for oc in range(2):
                for kc in range(2):
                    nc.tensor.matmul(
                        fps_[:, oc * 128:(oc + 1) * 128],
                        lhsT=w["finW"][:, kc * 256 + oc * 128:
                                       kc * 256 + oc * 128 + 128],
                        rhs=sv_r[:, kc * 128:kc * 128 + 128],
                        start=(kc == 0), stop=(kc == 1))
            fin_t = st.tile([128, 256], BF16, name="fin_t")   # holds s_f
            for oc in range(2):
                nc.scalar.activation(fin_t[:, oc * 128:(oc + 1) * 128],
                                     fps_[:, oc * 128:(oc + 1) * 128],
                                     AF.Sigmoid, bias=bcol("fin", oc))
            yp = gps.tile([1, BC], FP32, name="yp", tag="g")
            for kc in range(2):
                nc.tensor.matmul(yp[:, :], lhsT=w["w2pack"][:, 4 + kc:5 + kc],
                                 rhs=fin_t[:, kc * 128:kc * 128 + 128],
                                 start=(kc == 0), stop=(kc == 1))
            y_t = rowp.tile([1, BC], FP32, name="y_t", tag="y_t")
            nc.scalar.activation(y_t[:, :], yp[:, :], AF.Sigmoid,
                                 bias=bp[0:1, BCOL["pre"]:BCOL["pre"] + 1])
            nc.sync.dma_start(out=d["yout"][:, :], in_=y_t[:, :])

    if not nc.is_finalized():
        nc.finalize()
    return nc


_NC_CACHE = None
LAST_RESULTS = None
LAST_IN_MAPS = None


def _get_nc():
    global _NC_CACHE
    if _NC_CACHE is None:
        _NC_CACHE = build_nc()
    return _NC_CACHE


def _prep_weights(p):
    g = {k: np.asarray(v, np.float32) for k, v in p.items()}
    out = {}
    out["vggW"] = _pack(g["vgg_W"].T)
    out["emoW"] = _pack(g["emo_W"].T)
    scn = np.zeros((384, 256), np.float32)
    scn[:365] = g["scn_W"].T
    out["scnW"] = _pack(scn)
    out["posW"] = _pack(g["pos_W"].T)
    out["fusW"] = _pack(g["fus_W"].T)

    lf_cols, lf_b = _gate_reorder(
        np.vstack([g["lf_Wih"].T, g["lf_Whh"].T]), g["lf_b"])
    out["lfW"] = _pack(lf_cols)
    lb_cols, lb_b = _gate_reorder(g["lb_Wih"].T, g["lb_b"])
    out["lbW"] = _pack(lb_cols)
    out["brow"] = np.concatenate([lf_b, lb_b]).reshape(1, 2048)
    out["tmpW"] = _pack(g["tmp_W"].T)

    out["wr1s"] = _pack(g["wr1_W"][:, :256].T)
    out["wr1x"] = _pack(g["wr1_W"][:, 256:].T)
    out["fWs"] = _pack(g["f_W"][:, :256].T)
    out["fWx"] = _pack(g["f_W"][:, 256:].T)
    out["iWs"] = _pack(g["i_W"][:, :256].T)
    out["iWx"] = _pack(g["i_W"][:, 256:].T)
    # c gate: tanh -> sigmoid(2x)
    out["cWs"] = _pack(2.0 * g["c_W"][:, :256].T)
    out["cWx"] = _pack(2.0 * g["c_W"][:, 256:].T)
    # rd1: mem = 2s-1 folded: W' = 2W, b' = b - rowsum(W_mem_part)
    rd1s_o = g["rd1_W"][:, :256]
    out["rd1s"] = _pack(2.0 * rd1s_o.T)
    out["rd1x"] = _pack(g["rd1_W"][:, 256:].T)
    rd1_b_adj = g["rd1_b"] - rd1s_o.sum(axis=1)
    # sd / fin: tanh -> sigmoid(2x)
    out["sdW"] = _pack(2.0 * g["sd_W"].T)
    out["finW"] = _pack(2.0 * g["fin_W"].T)

    # pre: y = sig(pre@(2s-1)+b) = sig((2 pre)@s + b - sum(pre))
    pre_w = 2.0 * g["pre_W"].reshape(256)
    pre_b_adj = float(np.asarray(g["pre_b"]).reshape(-1)[0]) - float(
        g["pre_W"].reshape(256).sum())

    w2 = np.zeros((128, 6), np.float32)
    w2[:, 0:2] = g["wr2_W"].reshape(256).reshape(2, 128).T
    w2[:, 2:4] = g["rd2_W"].reshape(256).reshape(2, 128).T
    w2[:, 4:6] = pre_w.reshape(2, 128).T
    out["w2pack"] = w2

    sel = np.zeros((8, 512), np.float32)
    for b_ in range(8):
        sel[b_, b_ * 64:(b_ + 1) * 64] = 1.0
    out["sel8"] = sel
    out["qbrow"] = np.concatenate(
        [g["wr1_b"], g["f_b"], g["i_b"], 2.0 * g["c_b"],
         rd1_b_adj]).reshape(1, 1280)

    bpk = np.zeros((128, NBCOL), np.float32)
    for nm, vec in [("vgg", g["vgg_b"]), ("emo", g["emo_b"]),
                    ("scn", g["scn_b"]), ("pos", g["pos_b"]),
                    ("fus", g["fus_b"]), ("tmp", g["tmp_b"]),
                    ("wr1", g["wr1_b"]), ("f", g["f_b"]), ("i", g["i_b"]),
                    ("c", 2.0 * g["c_b"]), ("rd1", rd1_b_adj),
                    ("sd", 2.0 * g["sd_b"]), ("fin", 2.0 * g["fin_b"])]:
        bpk[:, BCOL[nm]:BCOL[nm] + 2] = np.asarray(vec).reshape(2, 128).T
    bpk[0, BCOL["pre"]] = pre_b_adj

    for k in out:
        out[k] = out[k].astype(BFNP)
    out["bpack"] = bpk
    return out


def kernel(vggish, bgm_emotion, emotion, scene, pose, sub_memory, target,
           params, epochs, training, **_unused):
    vggish = np.asarray(vggish, np.float32)
    emotion = np.asarray(emotion, np.float32)
    scene = np.asarray(scene, np.float32)
    pose = np.asarray(pose, np.float32)
    sub_memory = np.asarray(sub_memory, np.float32)

    wmaps = _prep_weights(params)
    nc = _get_nc()

    in_maps = []
    for ci in range(NCORES):
        bs = slice(ci * BC, (ci + 1) * BC)
        xin = np.empty((1536, COLS), BFNP)
        xin[0:128] = vggish[bs].transpose(2, 1, 0).reshape(128, COLS)
        xin[128:640] = emotion[bs].transpose(2, 1, 0).reshape(512, COLS)
        xin[640:1005] = scene[bs].transpose(2, 1, 0).reshape(365, COLS)
        xin[1005:1024] = 0.0
        xin[1024:1536] = pose[bs].transpose(2, 1, 0).reshape(512, COLS)
        sub = np.ascontiguousarray(
            sub_memory[bs].transpose(2, 0, 1).reshape(256, MCOLS)).astype(BFNP)
        m = {"xin": xin, "sub": sub, "ones": np.ones((1, 128), BFNP)}
        m.update(wmaps)
        in_maps.append(m)

    global LAST_RESULTS, LAST_IN_MAPS
    LAST_IN_MAPS = in_maps
    res = run_bass_kernel_spmd(nc, in_maps, list(range(NCORES)))
    LAST_RESULTS = res

    mem = np.empty((B, M, F), np.float32)
    y = np.empty((B, 1), np.float32)
    for ci in range(NCORES):
        bs = slice(ci * BC, (ci + 1) * BC)
        mo = res.results[ci]["mout"]            # fp32 sigmoid values
        mem[bs] = 2.0 * mo.reshape(256, BC, M).transpose(1, 2, 0) - 1.0
        y[bs] = res.results[ci]["yout"].reshape(BC, 1)
    return mem, y


if __name__ == "__main__":
    print("building nc...")
    nc = _get_nc()
    print("ok")


# revision 24
# speedup vs baseline: 6.2763x; 2.5525x over previous
"""Trainium2 Bass kernel for nn_Actor (scatter_memory).

Pure data parallel across 8 NeuronCores (128 samples each, no
collectives). Activations are feature-major ([feat_part, cols]); every
linear is lhsT=W.T chunks / rhs=activation chunks in bf16 (fp32 PSUM
accumulate). The LSTM uses batch-major gates (lhsT = x_t/h tiles,
rhs = weight rows); h returns to feature-major via PE transpose each
step. Memory-module columns are b-major (col = b*64 + m) so softmax
over M is local to each 512-column tile.

All ACT transcendentals use the sigmoid table: tanh(x) = 2*sigmoid(2x)-1
with the 2x folded into host-scaled weights and the affine folded into
neighbouring DVE ops or downstream weights (rd1/pre) or the host-side
output transform (mem = 2*s - 1).
"""

import os
import sys

import numpy as np
import ml_dtypes

for _p in ("/opt/trn_rl_repo",):
    if _p not in sys.path:
        sys.path.insert(0, _p)

os.environ.setdefault("JAX_PLATFORMS", "")

import concourse.bass as bass
import concourse.bacc as bacc
import concourse.mybir as mybir
import concourse.tile as tile
from concourse.bass_utils import run_bass_kernel_spmd
from concourse.masks import make_identity
from concourse.tile_rust import add_dep_helper

FP32 = mybir.dt.float32
BF16 = mybir.dt.bfloat16
AF = mybir.ActivationFunctionType
ALU = mybir.AluOpType
BFNP = ml_dtypes.bfloat16

B, T, M, F = 1024, 64, 64, 256
NCORES = 8
BC = B // NCORES          # 128 samples per core
COLS = T * BC             # phase-A columns (t-major: col = t*128 + b)
MCOLS = BC * M            # memory columns (b-major: col = b*64 + m)
NTILE = 512
NT_A = COLS // NTILE      # 16
NT_C = MCOLS // NTILE     # 16
SPT = 8                   # samples per 512-col memory tile

EF_BLOCKS = [
    ("v", 0, 1),    # vggish   rows 0:128
    ("e", 1, 4),    # emotion  rows 128:640
    ("s", 5, 3),    # scene    rows 640:1024 (365 padded to 384)
    ("p", 8, 4),    # pose     rows 1024:1536
]

BCOL = {}
_c = 0
for _n in ("vgg", "emo", "scn", "pos", "fus", "tmp", "wr1", "f", "i", "c",
           "rd1", "sd", "fin"):
    BCOL[_n] = _c
    _c += 2
BCOL["pre"] = _c
NBCOL = _c + 1


def _pack(wt):
    """[kin, out] (kin % 128 == 0) -> [128, (kin/128)*out]."""
    kin, out = wt.shape
    assert kin % 128 == 0
    return np.ascontiguousarray(
        wt.reshape(kin // 128, 128, out).transpose(1, 0, 2).reshape(128, -1))


def _gate_reorder(w_cols, b):
    """[i f g o] -> [i f o g]; double the g block (tanh-as-sigmoid)."""
    i, f, g, o = (w_cols[:, k * F:(k + 1) * F] for k in range(4))
    w = np.concatenate([i, 2.0 * g, f, o], axis=1)
    bi, bf, bg, bo = (b[k * F:(k + 1) * F] for k in range(4))
    return w, np.concatenate([bi, 2.0 * bg, bf, bo])


def build_nc():
    nc = bacc.Bacc(None)
    d = {}

    def param(name, shape, out=False, dt=FP32):
        d[name] = nc.declare_dram_parameter(name, list(shape), dt, isOutput=out)
        return d[name]

    param("xin", (1536, COLS), dt=BF16)
    param("sub", (F, MCOLS), dt=BF16)
    WSPEC = [("vggW", 256), ("emoW", 1024), ("scnW", 768), ("posW", 1024),
             ("fusW", 2048), ("lfW", 4096), ("lbW", 2048), ("tmpW", 1024),
             ("wr1s", 512), ("wr1x", 512), ("fWs", 512), ("fWx", 512),
             ("iWs", 512), ("iWx", 512), ("cWs", 512), ("cWx", 512),
             ("rd1s", 512), ("rd1x", 512), ("sdW", 512), ("finW", 512),
             ("w2pack", 6)]
    for n, c in WSPEC:
        param(n, (128, c), dt=BF16)
    param("bpack", (128, NBCOL))
    param("sel8", (8, NTILE), dt=BF16)
    param("qbrow", (1, 1280), dt=BF16)
    param("brow", (1, 2048), dt=BF16)
    param("ones", (1, 128), dt=BF16)
    param("mout", (F, MCOLS), out=True)
    param("yout", (1, BC), out=True)

    with tile.TileContext(nc) as tc:
        with (
            tc.tile_pool(name="wp", bufs=1) as wp,
            tc.tile_pool(name="state", bufs=1) as st,
            tc.tile_pool(name="efin", bufs=2) as efin,
            tc.tile_pool(name="actp", bufs=2) as actp,
            tc.tile_pool(name="fusedp", bufs=3) as fusedp,
            tc.tile_pool(name="lsp", bufs=2) as lsp,
            tc.tile_pool(name="subp", bufs=3) as subp,
            tc.tile_pool(name="cp", bufs=3) as cp,
            tc.tile_pool(name="rowp", bufs=2) as rowp,
            tc.tile_pool(name="efps", bufs=3, space="PSUM") as efps,
            tc.tile_pool(name="gps", bufs=2, space="PSUM") as gps,
            tc.tile_pool(name="tps", bufs=1, space="PSUM") as tps,
        ):
            # ---- weights ----------------------------------------------
            w = {}
            for n, c in WSPEC:
                w[n] = wp.tile([128, c], BF16, name=f"w_{n}")
                nc.sync.dma_start(out=w[n][:, :], in_=d[n][:, :])
            w["bpack"] = wp.tile([128, NBCOL], FP32, name="w_bpack")
            nc.sync.dma_start(out=w["bpack"][:, :], in_=d["bpack"][:, :])
            brow = wp.tile([1, 2048], BF16, name="w_brow")
            nc.sync.dma_start(out=brow[:, :], in_=d["brow"][:, :])
            sel8 = wp.tile([8, NTILE], BF16, name="w_sel8")
            nc.sync.dma_start(out=sel8[:, :], in_=d["sel8"][:, :])
            qbrow = wp.tile([1, 1280], BF16, name="w_qbrow")
            nc.sync.dma_start(out=qbrow[:, :], in_=d["qbrow"][:, :])
            ones1 = wp.tile([1, 128], BF16, name="ones1")
            nc.sync.dma_start(out=ones1[:, :], in_=d["ones"][:, :])
            ident = wp.tile([128, 128], FP32, name="ident")
            make_identity(nc, ident[:, :])
            ident2 = wp.tile([128, 128], FP32, name="ident2")
            make_identity(nc, ident2[:, :])
            nc.scalar.mul(ident2[:, :], ident2[:, :], 2.0)
            bp = w["bpack"]

            def bcol(name, oc=0):
                c0 = BCOL[name] + oc
                return bp[:, c0:c0 + 1]

            # ---- persistent state -------------------------------------
            hT = st.tile([128, 256], BF16, name="hT")
            c_bm = st.tile([128, 256], FP32, name="c_bm")
            nc.vector.memset(hT[:, :], 0.0)
            nc.vector.memset(c_bm[:, :], 0.0)
            sv_un = st.tile([128, 256], FP32, name="sv_un")
            s2row = st.tile([1, BC], FP32, name="s2row")

            # ============================================================
            # Phase A (early fusion) emitted in quarter-tile slices
            # between LSTM steps (fills PE during the step's serial
            # elementwise chain; the PE queue is in-order, so ready
            # matmuls must be emitted ahead of blocking step matmuls).
            # ============================================================
            WNAME = {"v": "vggW", "e": "emoW", "s": "scnW", "p": "posW"}
            BNAME = {"v": "vgg", "e": "emo", "s": "scn", "p": "pos"}

            def ef_input(n, stt_, name, blk0, nk, ei):
                xin_t = stt_["xin"]
                for oc in range(2):
                    ps = efps.tile([128, NTILE], FP32, name="ps_ef",
                                   tag="efps")
                    wt = w[WNAME[name]]
                    for kc in range(nk):
                        nc.tensor.matmul(
                            ps[:, :],
                            lhsT=wt[:, kc * 256 + oc * 128:
                                    kc * 256 + oc * 128 + 128],
                            rhs=xin_t[:, blk0 + kc],
                            start=(kc == 0), stop=(kc == nk - 1))
                    at = actp.tile([128, NTILE], BF16, name=f"act_{name}{oc}",
                                   tag=f"act_{name}{oc}")
                    bc = bcol(BNAME[name], oc)
                    if (ei + oc) % 2 == 0:
                        nc.scalar.activation(at[:, :], ps[:, :], AF.Relu,
                                             bias=bc)
                    else:
                        nc.vector.tensor_scalar(
                            out=at[:, :], in0=ps[:, :], scalar1=bc,
                            scalar2=0.0, op0=ALU.add, op1=ALU.max)
                    stt_["acts"][(name, oc)] = at

            gall_insts = []

            def ef_quarter(n, q, stt_):
                cs = slice(n * NTILE, (n + 1) * NTILE)
                if q == 0:
                    xin_t = efin.tile([128, 12, NTILE], BF16, name="xin_t")
                    stt_["xin"] = xin_t
                    stt_["acts"] = {}
                    pace = None
                    if n >= 4 and len(gall_insts) >= 8:
                        pace = gall_insts[4 * n - 10]
                    for j in range(12):
                        dma = nc.sync.dma_start(
                            out=xin_t[:, j],
                            in_=d["xin"][j * 128:(j + 1) * 128, cs])
                        if pace is not None and j < 6:
                            add_dep_helper(dma.ins, pace, sync=True,
                                           reason="pace phase A vs LSTM")
                    ef_input(n, stt_, "v", 0, 1, 0)
                elif q == 1:
                    ef_input(n, stt_, "e", 1, 4, 1)
                elif q == 2:
                    ef_input(n, stt_, "s", 5, 3, 0)
                    ef_input(n, stt_, "p", 8, 4, 1)
                else:
                    acts = stt_["acts"]
                    fused = []
                    for oc in range(2):
                        ps = efps.tile([128, NTILE], FP32, name="ps_fus",
                                       tag="efps")
                        mi = 0
                        for ai, (name, _, _) in enumerate(EF_BLOCKS):
                            for kc in range(2):
                                nc.tensor.matmul(
                                    ps[:, :],
                                    lhsT=w["fusW"][:, (ai * 2 + kc) * 256 + oc * 128:
                                                   (ai * 2 + kc) * 256 + oc * 128 + 128],
                                    rhs=acts[(name, kc)][:, :],
                                    start=(mi == 0), stop=(mi == 7))
                                mi += 1
                        ft = fusedp.tile([128, NTILE], BF16,
                                         name=f"fused{oc}", tag=f"fused{oc}")
                        if oc == 0:
                            nc.scalar.activation(ft[:, :], ps[:, :], AF.Relu,
                                                 bias=bcol("fus", oc))
                        else:
                            nc.vector.tensor_scalar(
                                out=ft[:, :], in0=ps[:, :],
                                scalar1=bcol("fus", oc), scalar2=0.0,
                                op0=ALU.add, op1=ALU.max)
                        fused.append(ft)
                    stt_["fused"] = fused

            def lstm_step(tt, fused):
                # gates (batch-major, cols reordered [i g' f o], g' = 2g):
                # s = sigmoid(gp); g~ = 2*s_g - 1
                # c = f*c + 2*(i*s_g) - i ;  h = 2*(o*sigmoid(2c)) - o
                t4 = tt * 128
                gp = gps.tile([128, 1024], FP32, name="gp", tag="g")
                for nh in range(2):
                    out_ap = gp[:, nh * 512:(nh + 1) * 512]
                    nc.tensor.matmul(out_ap, lhsT=ones1[:, :],
                                     rhs=brow[0:1, nh * 512:(nh + 1) * 512],
                                     start=True, stop=False)
                    for kc in range(4):
                        if kc < 2:
                            lhs = fused[kc][:, t4:t4 + 128]
                        else:
                            lhs = hT[:, (kc - 2) * 128:(kc - 1) * 128]
                        nc.tensor.matmul(
                            out_ap, lhsT=lhs,
                            rhs=w["lfW"][:, kc * 1024 + nh * 512:
                                         kc * 1024 + nh * 512 + 512],
                            start=False, stop=(kc == 3))
                gall = lsp.tile([128, 1024], FP32, name="gall", tag="gall")
                g_in = nc.scalar.activation(gall[:, 0:512], gp[:, 0:512],
                                            AF.Sigmoid)
                gall_insts.append(g_in.ins)
                nc.scalar.activation(gall[:, 512:1024], gp[:, 512:1024],
                                     AF.Sigmoid)
                gi, gs = gall[:, 0:256], gall[:, 256:512]
                gf, go = gall[:, 512:768], gall[:, 768:1024]
                # t1 = i*(s_g - 0.5); c = f*c + 2*t1
                t1 = lsp.tile([128, 256], FP32, name="t1", tag="t1")
                nc.vector.scalar_tensor_tensor(
                    out=t1[:, :], in0=gs, scalar=-0.5, in1=gi,
                    op0=ALU.add, op1=ALU.mult)
                nc.vector.tensor_mul(c_bm[:, :], c_bm[:, :], gf)
                nc.vector.scalar_tensor_tensor(
                    out=c_bm[:, :], in0=t1[:, :], scalar=2.0,
                    in1=c_bm[:, :], op0=ALU.mult, op1=ALU.add)
                sc = lsp.tile([128, 256], FP32, name="sc", tag="sc")
                nc.scalar.activation(sc[:, :], c_bm[:, :], AF.Sigmoid,
                                     scale=2.0)
                # h = 2*o*(s_c-0.5); the 2x rides the transpose (ident2)
                h_bm = lsp.tile([128, 256], FP32, name="h_bm", tag="h_bm")
                nc.vector.scalar_tensor_tensor(
                    out=h_bm[:, :], in0=sc[:, :], scalar=-0.5, in1=go,
                    op0=ALU.add, op1=ALU.mult)
                tp = tps.tile([128, 256], FP32, name="tp", tag="tp")
                nc.tensor.transpose(tp[:, 0:128], h_bm[:, 0:128], ident2[:, :])
                nc.tensor.transpose(tp[:, 128:256], h_bm[:, 128:256],
                                    ident2[:, :])
                nc.scalar.copy(hT[:, 0:128], tp[:, 0:128])
                nc.vector.tensor_copy(hT[:, 128:256], tp[:, 128:256])

            st_cur = {}
            for q in range(4):
                ef_quarter(0, q, st_cur)
            fused_cur = st_cur["fused"]
            st_nxt = {}
            for t in range(T):
                lstm_step(t % 4, fused_cur)
                n_next = t // 4 + 1
                if n_next < NT_A:
                    ef_quarter(n_next, t % 4, st_nxt)
                    if t % 4 == 3:
                        fused_cur = st_nxt["fused"]
                        st_nxt = {}
            last_fused = fused_cur

            # ============================================================
            # backward cell (t = T-1), then x = relu(tmp([h_fwd; h_bwd]))
            # ============================================================
            gp = gps.tile([128, 1024], FP32, name="gp_b", tag="g")
            for nh in range(2):
                out_ap = gp[:, nh * 512:(nh + 1) * 512]
                nc.tensor.matmul(out_ap, lhsT=ones1[:, :],
                                 rhs=brow[0:1, 1024 + nh * 512:
                                          1024 + (nh + 1) * 512],
                                 start=True, stop=False)
                for kc in range(2):
                    nc.tensor.matmul(
                        out_ap, lhsT=last_fused[kc][:, 384:512],
                        rhs=w["lbW"][:, kc * 1024 + nh * 512:
                                     kc * 1024 + nh * 512 + 512],
                        start=False, stop=(kc == 1))
            gall = lsp.tile([128, 1024], FP32, name="gall_b", tag="gall")
            nc.scalar.activation(gall[:, :], gp[:, :], AF.Sigmoid)
            gi, gs, go = gall[:, 0:256], gall[:, 256:512], gall[:, 768:1024]
            t1 = lsp.tile([128, 256], FP32, name="t1b", tag="t1")
            nc.vector.scalar_tensor_tensor(
                out=t1[:, :], in0=gs, scalar=-0.5, in1=gi,
                op0=ALU.add, op1=ALU.mult)
            scb = lsp.tile([128, 256], FP32, name="scb", tag="t2")
            nc.scalar.activation(scb[:, :], t1[:, :], AF.Sigmoid, scale=4.0)
            hb = lsp.tile([128, 256], FP32, name="hb", tag="h_bm")
            nc.vector.scalar_tensor_tensor(
                out=hb[:, :], in0=scb[:, :], scalar=-0.5, in1=go,
                op0=ALU.add, op1=ALU.mult)
            tp = tps.tile([128, 256], FP32, name="tp_b", tag="tp")
            nc.tensor.transpose(tp[:, 0:128], hb[:, 0:128], ident2[:, :])
            nc.tensor.transpose(tp[:, 128:256], hb[:, 128:256], ident2[:, :])
            hbT = st.tile([128, 256], BF16, name="hbT")
            nc.scalar.copy(hbT[:, 0:128], tp[:, 0:128])
            nc.vector.tensor_copy(hbT[:, 128:256], tp[:, 128:256])

            # x (feature-major [256, 128]) = relu(tmpW.T @ [hT; hbT] + b)
            xps = tps.tile([128, 256], FP32, name="xps", tag="tp")
            for oc in range(2):
                for kc in range(4):
                    rhs = hT[:, kc * 128:kc * 128 + 128] if kc < 2 else \
                        hbT[:, (kc - 2) * 128:(kc - 2) * 128 + 128]
                    nc.tensor.matmul(
                        xps[:, oc * 128:(oc + 1) * 128],
                        lhsT=w["tmpW"][:, kc * 256 + oc * 128:
                                       kc * 256 + oc * 128 + 128],
                        rhs=rhs, start=(kc == 0), stop=(kc == 3))
            x_fm = st.tile([128, 256], BF16, name="x_fm")
            for oc in range(2):
                nc.scalar.activation(x_fm[:, oc * 128:(oc + 1) * 128],
                                     xps[:, oc * 128:(oc + 1) * 128],
                                     AF.Relu, bias=bcol("tmp", oc))

            # q_bm[b, out] = x @ Wx.T + b  (batch-major, bias via ones-row;
            # per-tile 8-row slices feed K=8 selector matmuls)
            q_bm = {}
            for qi, (qn, wn) in enumerate([("qwr", "wr1x"), ("qf", "fWx"),
                                           ("qi", "iWx"), ("qc", "cWx"),
                                           ("qrd", "rd1x")]):
                ps = tps.tile([128, 256], FP32, name=f"ps_{qn}", tag="tp")
                nc.tensor.matmul(ps[:, :], lhsT=ones1[:, :],
                                 rhs=qbrow[0:1, qi * 256:(qi + 1) * 256],
                                 start=True, stop=False)
                for kc in range(2):
                    nc.tensor.matmul(
                        ps[:, :], lhsT=x_fm[:, kc * 128:kc * 128 + 128],
                        rhs=w[wn][:, kc * 256:(kc + 1) * 256],
                        start=False, stop=(kc == 1))
                q = st.tile([128, 256], BF16, name=f"qbm_{qn}")
                nc.scalar.copy(q[:, :], ps[:, :])
                q_bm[qi] = q

            # ============================================================
            # Memory module: 16 tiles x 512 cols (8 samples each)
            # ============================================================
            for n in range(NT_C):
                cs = slice(n * NTILE, (n + 1) * NTILE)
                sub_t = subp.tile([128, 2, NTILE], BF16, name="sub_t",
                                  tag="sub")
                for kc in range(2):
                    nc.sync.dma_start(out=sub_t[:, kc],
                                      in_=d["sub"][kc * 128:(kc + 1) * 128, cs])
                qrow8 = cp.tile([8, 5, 256], BF16, name="qrow8", tag="qrow")
                for qi in range(5):
                    nc.sync.dma_start(out=qrow8[:, qi],
                                      in_=q_bm[qi][n * SPT:(n + 1) * SPT, :])

                def cat_linear(ws_name, qi, act, bufname):
                    outs = []
                    odt = BF16 if act == "relu" else FP32
                    for oc in range(2):
                        ps = efps.tile([128, NTILE], FP32, name=f"ps_{bufname}",
                                       tag="efps")
                        for kc in range(2):
                            nc.tensor.matmul(
                                ps[:, :],
                                lhsT=w[ws_name][:, kc * 256 + oc * 128:
                                                kc * 256 + oc * 128 + 128],
                                rhs=sub_t[:, kc],
                                start=(kc == 0), stop=False)
                        nc.tensor.matmul(
                            ps[:, :],
                            lhsT=qrow8[:, qi, oc * 128:oc * 128 + 128],
                            rhs=sel8[:, :], start=False, stop=True)
                        ot = cp.tile([128, NTILE], odt, name=f"{bufname}{oc}",
                                     tag=f"{bufname}{oc}")
                        if act == "relu":
                            if oc == 0:
                                nc.scalar.activation(ot[:, :], ps[:, :],
                                                     AF.Relu)
                            else:
                                nc.vector.tensor_scalar(
                                    out=ot[:, :], in0=ps[:, :],
                                    scalar1=0.0, scalar2=None, op0=ALU.max)
                        else:
                            nc.scalar.activation(ot[:, :], ps[:, :],
                                                 AF.Sigmoid)
                        outs.append(ot)
                    return outs

                r1 = cat_linear("wr1s", 0, "relu", "r1")
                zp = gps.tile([1, NTILE], FP32, name="zp", tag="g")
                for kc in range(2):
                    nc.tensor.matmul(zp[:, :], lhsT=w["w2pack"][:, kc:kc + 1],
                                     rhs=r1[kc][:, :],
                                     start=(kc == 0), stop=(kc == 1))
                ex1 = rowp.tile([1, NTILE], FP32, name="ex1", tag="ex1")
                exn = rowp.tile([1, NTILE], FP32, name="exn", tag="exn")
                nc.scalar.activation(ex1[:, :], zp[:, :], AF.Sigmoid)
                nc.scalar.activation(exn[:, :], zp[:, :], AF.Sigmoid,
                                     scale=-1.0)
                nc.vector.reciprocal(exn[:, :], exn[:, :])
                nc.vector.tensor_mul(ex1[:, :], ex1[:, :], exn[:, :])
                s1 = rowp.tile([1, SPT], FP32, name="s1", tag="s1")
                nc.vector.tensor_reduce(
                    s1[:, :], ex1[:, :].rearrange("p (b m) -> p b m", b=SPT),
                    axis=mybir.AxisListType.X, op=ALU.add)
                rc1 = rowp.tile([1, SPT], FP32, name="rc1", tag="rc1")
                nc.vector.reciprocal(rc1[:, :], s1[:, :])
                nc.vector.tensor_mul(
                    ex1[:, :].rearrange("p (b m) -> p b m", b=SPT),
                    ex1[:, :].rearrange("p (b m) -> p b m", b=SPT),
                    rc1[:, :].unsqueeze(2).broadcast_to([1, SPT, M]))
                simb_t = cp.tile([128, NTILE], FP32, name="simb_t",
                                 tag="bcast")
                nc.gpsimd.partition_broadcast(simb_t[:, :], ex1[:, :])
                simb = simb_t[:, :]

                f_t = cat_linear("fWs", 1, "sig", "ft")
                i_t = cat_linear("iWs", 2, "sig", "it")
                c_t = cat_linear("cWs", 3, "sig", "ct")   # holds s_c

                # mem_pre = sub - sim*(f*sub + i*(1 - 2*s_c))   (in place)
                for oc in range(2):
                    nc.gpsimd.tensor_scalar(
                        out=c_t[oc][:, :], in0=c_t[oc][:, :], scalar1=-2.0,
                        scalar2=1.0, op0=ALU.mult, op1=ALU.add)
                    nc.gpsimd.tensor_mul(c_t[oc][:, :], c_t[oc][:, :],
                                         i_t[oc][:, :])
                    nc.vector.tensor_mul(f_t[oc][:, :], f_t[oc][:, :],
                                         sub_t[:, oc])
                    nc.vector.tensor_add(f_t[oc][:, :], f_t[oc][:, :],
                                         c_t[oc][:, :])
                    nc.gpsimd.tensor_mul(f_t[oc][:, :], f_t[oc][:, :], simb)
                    nc.vector.tensor_sub(sub_t[:, oc], sub_t[:, oc],
                                         f_t[oc][:, :])

                # mem_s = sigmoid(2*(sd@mem_pre) + 2*sd_b); mem = 2*mem_s-1
                # (weights pre-doubled on host; rd1/pre folded downstream;
                #  fp32 mem_s DMA'd out, host applies 2s-1)
                mem_s, mem_b = [], []
                for oc in range(2):
                    ps = efps.tile([128, NTILE], FP32, name="ps_sd",
                                   tag="efps")
                    for kc in range(2):
                        nc.tensor.matmul(
                            ps[:, :],
                            lhsT=w["sdW"][:, kc * 256 + oc * 128:
                                          kc * 256 + oc * 128 + 128],
                            rhs=sub_t[:, kc],
                            start=(kc == 0), stop=(kc == 1))
                    ms = cp.tile([128, NTILE], FP32, name=f"mems{oc}",
                                 tag=f"mems{oc}")
                    nc.scalar.activation(ms[:, :], ps[:, :], AF.Sigmoid,
                                         bias=bcol("sd", oc))
                    nc.sync.dma_start(
                        out=d["mout"][oc * 128:(oc + 1) * 128, cs],
                        in_=ms[:, :])
                    mb = cp.tile([128, NTILE], BF16, name=f"memb{oc}",
                                 tag=f"memb{oc}")
                    if oc == 0:
                        nc.scalar.copy(mb[:, :], ms[:, :])
                    else:
                        nc.vector.tensor_copy(mb[:, :], ms[:, :])
                    mem_s.append(ms)
                    mem_b.append(mb)

                # read attention (rd1s/q_rd host-folded for mem = 2s-1)
                r2 = []
                for oc in range(2):
                    ps = efps.tile([128, NTILE], FP32, name="ps_r2",
                                   tag="efps")
                    for kc in range(2):
                        nc.tensor.matmul(
                            ps[:, :],
                            lhsT=w["rd1s"][:, kc * 256 + oc * 128:
                                           kc * 256 + oc * 128 + 128],
                            rhs=mem_b[kc][:, :],
                            start=(kc == 0), stop=False)
                    nc.tensor.matmul(
                        ps[:, :], lhsT=qrow8[:, 4, oc * 128:oc * 128 + 128],
                        rhs=sel8[:, :], start=False, stop=True)
                    ot = cp.tile([128, NTILE], BF16, name=f"r2{oc}",
                                 tag=f"r1{oc}")
                    if oc == 0:
                        nc.scalar.activation(ot[:, :], ps[:, :], AF.Relu)
                    else:
                        nc.vector.tensor_scalar(
                            out=ot[:, :], in0=ps[:, :], scalar1=0.0,
                            scalar2=None, op0=ALU.max)
                    r2.append(ot)
                zp2 = gps.tile([1, NTILE], FP32, name="zp2", tag="g")
                for kc in range(2):
                    nc.tensor.matmul(zp2[:, :],
                                     lhsT=w["w2pack"][:, 2 + kc:3 + kc],
                                     rhs=r2[kc][:, :],
                                     start=(kc == 0), stop=(kc == 1))
                ex2 = rowp.tile([1, NTILE], FP32, name="ex2", tag="ex1")
                ex2n = rowp.tile([1, NTILE], FP32, name="ex2n", tag="exn")
                nc.scalar.activation(ex2[:, :], zp2[:, :], AF.Sigmoid)
                nc.scalar.activation(ex2n[:, :], zp2[:, :], AF.Sigmoid,
                                     scale=-1.0)
                nc.vector.reciprocal(ex2n[:, :], ex2n[:, :])
                nc.vector.tensor_mul(ex2[:, :], ex2[:, :], ex2n[:, :])
                nc.vector.tensor_reduce(
                    s2row[:, n * SPT:(n + 1) * SPT],
                    ex2[:, :].rearrange("p (b m) -> p b m", b=SPT),
                    axis=mybir.AxisListType.X, op=ALU.add)
                ex2b_t = cp.tile([128, NTILE], FP32, name="ex2b_t",
                                 tag="bcast")
                nc.gpsimd.partition_broadcast(ex2b_t[:, :], ex2[:, :])
                # P = sum_m s*e2; sub_vec = 2*P/S2 - 1 (affine at the tail)
                for oc in range(2):
                    wm = f_t[oc]
                    nc.gpsimd.tensor_mul(wm[:, :], mem_s[oc][:, :],
                                         ex2b_t[:, :])
                    nc.vector.tensor_reduce(
                        sv_un[:, oc * 128 + n * SPT: oc * 128 + (n + 1) * SPT],
                        wm[:, :].rearrange("p (b m) -> p b m", b=SPT),
                        axis=mybir.AxisListType.X, op=ALU.add)

            # ---- tail: sub_vec, final, y ------------------------------
            rc2 = rowp.tile([1, BC], FP32, name="rc2", tag="rc2")
            nc.vector.reciprocal(rc2[:, :], s2row[:, :])
            nc.vector.tensor_scalar(out=rc2[:, :], in0=rc2[:, :],
                                    scalar1=2.0, scalar2=None, op0=ALU.mult)
            rc2b_t = cp.tile([128, BC], FP32, name="rc2b_t", tag="bcast")
            nc.gpsimd.partition_broadcast(rc2b_t[:, :], rc2[:, :])
            sv_r = st.tile([128, 256], BF16, name="sv_r")
            for oc in range(2):
                nc.vector.tensor_mul(sv_un[:, oc * 128:(oc + 1) * 128],
                                     sv_un[:, oc * 128:(oc + 1) * 128],
                                     rc2b_t[:, :])
                nc.vector.tensor_scalar(
                    out=sv_r[:, oc * 128:(oc + 1) * 128],
                    in0=sv_un[:, oc * 128:(oc + 1) * 128],
                    scalar1=-1.0, scalar2=None, op0=ALU.add)
            fps_ = tps.tile([128, 256], FP32, name="fps", tag="tp")
            for oc in range(2):
                for kc in range(2):
                    nc.tensor.matmul(
                        fps_[:, oc * 128:(oc + 1) * 128],
                        lhsT=w["finW"][:, kc * 256 + oc * 128:
                                       kc * 256 + oc * 128 + 128],
                        rhs=sv_r[:, kc * 128:kc * 128 + 128],
                        start=(kc == 0), stop=(kc == 1))
            fin_t = st.tile([128, 256], BF16, name="fin_t")   # holds s_f
            for oc in range(2):
                nc.scalar.activation(fin_t[:, oc * 128:(oc + 1) * 128],
                                     fps_[:, oc * 128:(oc + 1) * 128],
                                     AF.Sigmoid, bias=bcol("fin", oc))
            yp = gps.tile([1, BC], FP32, name="yp", tag="g")
            for kc in range(2):
                nc.tensor.matmul(yp[:, :], lhsT=w["w2pack"][:, 4 + kc:5 + kc],
                                 rhs=fin_t[:, kc * 128:kc * 128 + 128],
                                 start=(kc == 0), stop=(kc == 1))
            y_t = rowp.tile([1, BC], FP32, name="y_t", tag="y_t")
            nc.scalar.activation(y_t[:, :], yp[:, :], AF.Sigmoid,
                                 bias=bp[0:1, BCOL["pre"]:BCOL["pre"] + 1])
            nc.sync.dma_start(out=d["yout"][:, :], in_=y_t[:, :])

    if not nc.is_finalized():
        nc.finalize()
    return nc


_NC_CACHE = None
LAST_RESULTS = None
LAST_IN_MAPS = None


def _get_nc():
    global _NC_CACHE
    if _NC_CACHE is None:
        _NC_CACHE = build_nc()
    return _NC_CACHE


def _prep_weights(p):
    g = {k: np.asarray(v, np.float32) for k, v in p.items()}
    out = {}
    out["vggW"] = _pack(g["vgg_W"].T)
    out["emoW"] = _pack(g["emo_W"].T)
    scn = np.zeros((384, 256), np.float32)
    scn[:365] = g["scn_W"].T
    out["scnW"] = _pack(scn)
    out["posW"] = _pack(g["pos_W"].T)
    out["fusW"] = _pack(g["fus_W"].T)

    lf_cols, lf_b = _gate_reorder(
        np.vstack([g["lf_Wih"].T, g["lf_Whh"].T]), g["lf_b"])
    out["lfW"] = _pack(lf_cols)
    lb_cols, lb_b = _gate_reorder(g["lb_Wih"].T, g["lb_b"])
    out["lbW"] = _pack(lb_cols)
    out["brow"] = np.concatenate([lf_b, lb_b]).reshape(1, 2048)
    out["tmpW"] = _pack(g["tmp_W"].T)

    out["wr1s"] = _pack(g["wr1_W"][:, :256].T)
    out["wr1x"] = _pack(g["wr1_W"][:, 256:].T)
    out["fWs"] = _pack(g["f_W"][:, :256].T)
    out["fWx"] = _pack(g["f_W"][:, 256:].T)
    out["iWs"] = _pack(g["i_W"][:, :256].T)
    out["iWx"] = _pack(g["i_W"][:, 256:].T)
    # c gate: tanh -> sigmoid(2x)
    out["cWs"] = _pack(2.0 * g["c_W"][:, :256].T)
    out["cWx"] = _pack(2.0 * g["c_W"][:, 256:].T)
    # rd1: mem = 2s-1 folded: W' = 2W, b' = b - rowsum(W_mem_part)
    rd1s_o = g["rd1_W"][:, :256]
    out["rd1s"] = _pack(2.0 * rd1s_o.T)
    out["rd1x"] = _pack(g["rd1_W"][:, 256:].T)
    rd1_b_adj = g["rd1_b"] - rd1s_o.sum(axis=1)
    # sd / fin: tanh -> sigmoid(2x)
    out["sdW"] = _pack(2.0 * g["sd_W"].T)
    out["finW"] = _pack(2.0 * g["fin_W"].T)

    # pre: y = sig(pre@(2s-1)+b) = sig((2 pre)@s + b - sum(pre))
    pre_w = 2.0 * g["pre_W"].reshape(256)
    pre_b_adj = float(np.asarray(g["pre_b"]).reshape(-1)[0]) - float(
        g["pre_W"].reshape(256).sum())

    w2 = np.zeros((128, 6), np.float32)
    w2[:, 0:2] = g["wr2_W"].reshape(256).reshape(2, 128).T
    w2[:, 2:4] = g["rd2_W"].reshape(256).reshape(2, 128).T
    w2[:, 4:6] = pre_w.reshape(2, 128).T
    out["w2pack"] = w2

    sel = np.zeros((8, 512), np.float32)
    for b_ in range(8):
        sel[b_, b_ * 64:(b_ + 1) * 64] = 1.0
    out["sel8"] = sel
    out["qbrow"] = np.concatenate(
        [g["wr1_b"], g["f_b"], g["i_b"], 2.0 * g["c_b"],
         rd1_b_adj]).reshape(1, 1280)

    bpk = np.zeros((128, NBCOL), np.float32)
    for nm, vec in [("vgg", g["vgg_b"]), ("emo", g["emo_b"]),
                    ("scn", g["scn_b"]), ("pos", g["pos_b"]),
                    ("fus", g["fus_b"]), ("tmp", g["tmp_b"]),
                    ("wr1", g["wr1_b"]), ("f", g["f_b"]), ("i", g["i_b"]),
                    ("c", 2.0 * g["c_b"]), ("rd1", rd1_b_adj),
                    ("sd", 2.0 * g["sd_b"]), ("fin", 2.0 * g["fin_b"])]:
        bpk[:, BCOL[nm]:BCOL[nm] + 2] = np.asarray(vec).reshape(2, 128).T
    bpk[0, BCOL["pre"]] = pre_b_adj

    for k in out:
        out[k] = out[k].astype(BFNP)
    out["bpack"] = bpk
    return out


def kernel(vggish, bgm_emotion, emotion, scene, pose, sub_memory, target,
           params, epochs, training, **_unused):
    vggish = np.asarray(vggish, np.float32)
    emotion = np.asarray(emotion, np.float32)
    scene = np.asarray(scene, np.float32)
    pose = np.asarray(pose, np.float32)
    sub_memory = np.asarray(sub_memory, np.float32)

    wmaps = _prep_weights(params)
    nc = _get_nc()

    in_maps = []
    for ci in range(NCORES):
        bs = slice(ci * BC, (ci + 1) * BC)
        xin = np.empty((1536, COLS), BFNP)
        xin[0:128] = vggish[bs].transpose(2, 1, 0).reshape(128, COLS)
        xin[128:640] = emotion[bs].transpose(2, 1, 0).reshape(512, COLS)
        xin[640:1005] = scene[bs].transpose(2, 1, 0).reshape(365, COLS)
        xin[1005:1024] = 0.0
        xin[1024:1536] = pose[bs].transpose(2, 1, 0).reshape(512, COLS)
        sub = np.ascontiguousarray(
            sub_memory[bs].transpose(2, 0, 1).reshape(256, MCOLS)).astype(BFNP)
        m = {"xin": xin, "sub": sub, "ones": np.ones((1, 128), BFNP)}
        m.update(wmaps)
        in_maps.append(m)

    global LAST_RESULTS, LAST_IN_MAPS
    LAST_IN_MAPS = in_maps
    res = run_bass_kernel_spmd(nc, in_maps, list(range(NCORES)))
    LAST_RESULTS = res

    mem = np.empty((B, M, F), np.float32)
    y = np.empty((B, 1), np.float32)
    for ci in range(NCORES):
        bs = slice(ci * BC, (ci + 1) * BC)
        mo = res.results[ci]["mout"]            # fp32 sigmoid values
        mem[bs] = 2.0 * mo.reshape(256, BC, M).transpose(1, 2, 0) - 1.0
        y[bs] = res.results[ci]["yout"].reshape(BC, 1)
    return mem, y


if __name__ == "__main__":
    print("building nc...")
    nc = _get_nc()
    print("ok")
